# revision 18
# baseline (speedup 1.0000x reference)
"""Trainium2 Bass kernel for nn_AttentionBlock (B=2, S=2048, D=1024, H=16, HD=64).

Sharding: 8 cores = 2 batches x 4 head-groups (4 heads each).
Each core computes, for its (batch b, head-group g):
  - fused QK projection (RoPE'd, feature-transposed layout) + V projection
  - causal attention for its 4 heads (scores computed transposed, softmax
    denominator via an appended ones-column in the PV matmul)
  - a PARTIAL output projection: x_g @ W_out[:, d-slice].T  -> [S, D] partial
The host sums the 4 partials per batch (linear unshard step) - no on-device
collectives needed.

Schedule: phase A runs the QKV projections for all four 512-column chunks
(V of chunk 3 deferred), then attention quarters are processed in order
(0, 3, 2, 1).  Within a quarter the t-loop is software-pipelined
(scores(t) -> exp(t) -> PV(t-1)); both heads of a pair share one 2-bank
PSUM score tile so a single exp instruction covers them; causal masking is
done inside the scores matmul group (a -1e8 upper-triangular add) so no
vector-engine op sits on the critical chain.  The out-projection of each
finished quarter and the deferred V tiles are emitted as PE "filler"
pieces inside the next quarter's loop, positioned in the trailing diagonal
region where the ACT engine otherwise outpaces the PE; each pair's final
filler slot also pre-starts the NEXT pair's first score tile + exp so the
ACT pipeline stays warm across seams.
PSUM: projections 6+2 banks; attention 4 (scores) + 2 (xacc) + 2 (outproj).

Dtypes: the QKV projections run as fp8(e4m3) DoubleRow matmuls on hi/lo
split operands (x*8, W_in*32: Wh(xh+xl) + Wl xh), which both doubles PE
throughput and carries ~11 effective mantissa bits; everything downstream
(RoPE output, scores, pt, v, xn, W_out) is bf16 with fp32 PSUM
accumulation; output partials are written bf16 (rescaled by 1/256).

Self-contained: hardcodes all shapes; imports only concourse + numpy.
"""
import math

import numpy as np

import concourse.bass as bass  # noqa: F401
import concourse.bacc as bacc
import concourse.mybir as mybir
import concourse.tile as tile
from concourse import bass_utils
from concourse.masks import make_identity, make_upper_triangular

B, S, D, H = 2, 2048, 1024, 16
HD = D // H            # 64
G = 4                  # head-groups (cores per batch)
HPG = H // G           # 4 heads per group
N_CORES = 8
ROPE_BASE = 10000.0
F32 = mybir.dt.float32
F32R = mybir.dt.float32r
BF16 = mybir.dt.bfloat16
F8 = mybir.dt.float8e4
DR = mybir.MatmulPerfMode.DoubleRow

KT = S // 128          # 16 k-tiles of 128
ST = S // 128          # 16 s-tiles
DT = D // 128          # 8 d-chunks
X_SCALE = 8.0          # fp8 pre-scale on x
W_SCALE = 32.0         # fp8 pre-scale on W_in (lifts W out of e4m3 subnormals)
QK_SCALE = (X_SCALE * W_SCALE) ** 2   # scores carry (x*W)^2 scaling
V_SCALE = X_SCALE * W_SCALE           # v / xn / out-proj psum carry this
SCALE = 1.0 / math.sqrt(HD)


def build_nc():
    nc = bacc.Bacc("TRN2", target_bir_lowering=False, debug=False,
                   num_devices=N_CORES)

    # x (x8) and projection weights (x32) in fp8 e4m3 hi/lo pairs: the
    # projections run as DoubleRow fp8 matmuls (Wh(xh+xl) + Wl xh), which
    # stream two contraction rows per cycle on the PE.
    xT = nc.dram_tensor("xT", [D, 2, S], F8, kind="ExternalInput").ap()
    # QK weights, transposed+permuted: columns = [QA|QB|KA|KB] of 128 each.
    wqkT = nc.dram_tensor("wqkT", [D, 2, 512], F8, kind="ExternalInput").ap()
    # V weights, transposed: columns = 4 heads x 64 feats.
    wvT = nc.dram_tensor("wvT", [D, 2, 256], F8, kind="ExternalInput").ap()
    # RoPE tables, [128, S]: rows = 4x (32 freqs).
    cos4 = nc.dram_tensor("cos4", [128, S], F32, kind="ExternalInput").ap()
    sin4 = nc.dram_tensor("sin4", [128, S], F32, kind="ExternalInput").ap()
    # W_out columns for this group's features, transposed: [256, D].
    woT = nc.dram_tensor("woT", [256, D], BF16, kind="ExternalInput").ap()
    out = nc.dram_tensor("out", [S, D], BF16, kind="ExternalOutput").ap()

    with tile.TileContext(nc) as tc:
        _body(nc, tc, xT, wqkT, wvT, cos4, sin4, woT, out)
    nc.compile()
    return nc


def _outproj_units(nc, opsp, ooutp, xn, wo_t, out, qh, tail=False,
                   act_c1=False):
    """Yield closures, each emitting one (st, c) out-projection piece
    (2 matmuls + psum->sbuf copy + dma on the second half).  Used as PE
    filler work.  The tail variant allocates 2-bank psum tiles (from the
    then-idle attention score pool) so both halves run concurrently, and
    puts half the copies on ACT so the final drain parallelizes."""
    state = {}
    pstate = {}

    def unit(st, c):
        stl = slice(st * 128, (st + 1) * 128)
        if st not in state:
            state[st] = ooutp.tile([128, 1024], BF16, tag="ot",
                                   name=f"ot_{qh}_{st}")
        ot = state[st]
        if tail:
            if st not in pstate:
                pstate[st] = opsp.tile([128, 1024], F32, tag="sc",
                                       name=f"pot_{qh}_{st}")
            po = pstate[st][:, c * 512:(c + 1) * 512]
        else:
            po = opsp.tile([128, 512], F32, tag="ops",
                           name=f"po_{qh}_{st}_{c}")
        for d2 in range(2):
            nc.tensor.matmul(
                po, xn[d2][:, stl], wo_t[d2][:, c * 512:(c + 1) * 512],
                start=(d2 == 0), stop=(d2 == 1))
        if c == 1 and (tail or act_c1):
            # tail copies on ACT so the final drain parallelizes with DVE
            nc.scalar.mul(ot[:, c * 512:(c + 1) * 512], po, 1.0 / V_SCALE)
        else:
            nc.vector.tensor_scalar_mul(ot[:, c * 512:(c + 1) * 512], po,
                                        1.0 / V_SCALE)
        nc.sync.dma_start(out[stl, c * 512:(c + 1) * 512],
                          ot[:, c * 512:(c + 1) * 512])

    cs = (1, 0) if tail else (0, 1)
    for st in range(4 * qh, 4 * qh + 4):
        for c in cs:
            yield (lambda st=st, c=c: unit(st, c))


def _tail_outproj(nc, attps, opsp, ooutp, xn, wo_t, out):
    """Two-part tail for quarter 1 (st 4..7): part1 emits the d2=0 halves of
    st4/st5 (xn[0]-only dependency, runnable during the final norm ladder);
    part2 closes them and runs st6/st7, copies split ACT(c1)/DVE(c0).
    st6 borrows the (tail-idle) ops pool so it doesn't wait on st4/st5's
    sc-slot copies."""
    state = {}
    pstate = {}

    def ensure(st):
        if st not in state:
            state[st] = ooutp.tile([128, 1024], BF16, tag="ot",
                                   name=f"ot_t_{st}")
            if st == 6:
                pstate[st] = [opsp.tile([128, 512], F32, tag="ops",
                                        name=f"pot_t_{st}_{c}")
                              for c in range(2)]
            else:
                pt_ = attps.tile([128, 1024], F32, tag="sc",
                                 name=f"pot_t_{st}")
                pstate[st] = [pt_[:, 0:512], pt_[:, 512:1024]]

    def mm(st, c, d2, start, stop):
        stl = slice(st * 128, (st + 1) * 128)
        po = pstate[st][c]
        nc.tensor.matmul(po, xn[d2][:, stl],
                         wo_t[d2][:, c * 512:(c + 1) * 512],
                         start=start, stop=stop)

    def finish(st, c):
        stl = slice(st * 128, (st + 1) * 128)
        po = pstate[st][c]
        ot = state[st]
        if c == 1:
            nc.scalar.mul(ot[:, c * 512:(c + 1) * 512], po, 1.0 / V_SCALE)
        else:
            nc.vector.tensor_scalar_mul(ot[:, c * 512:(c + 1) * 512], po,
                                        1.0 / V_SCALE)
            # tail units run c=1 then c=0: one full-width DMA (single
            # hwdge slot) after the second copy shortens the final drain
            nc.sync.dma_start(out[stl, :], ot[:])

    def part1():
        for st in (4, 5):
            ensure(st)
            for c in (0, 1):
                mm(st, c, 0, True, False)

    def part2():
        for st in (4, 5):
            for c in (1, 0):
                mm(st, c, 1, False, True)
                finish(st, c)
        for st in (6, 7):
            ensure(st)
            for c in (1, 0):
                mm(st, c, 0, True, False)
                mm(st, c, 1, False, True)
                finish(st, c)

    return part1, part2


def _v_pieces(nc, sts, v_alloc, x_t, wv_t, v_t):
    """Yield closures emitting the V-projection of s-tiles `sts`, two pieces
    (6 fp8 DoubleRow matmuls) each."""
    ps = {}
    TERMS = ((0, 0), (0, 1), (1, 0))  # (W hi/lo, x hi/lo)

    def v_piece(st, half):
        if half == 0:
            ps[st] = v_alloc(st)
        pv = ps[st]
        stl = slice(st * 128, (st + 1) * 128)
        for i in range(6 * half, 6 * half + 6):
            (wh, xh), dp = TERMS[i // 4], i % 4
            nc.tensor.matmul(
                pv[:, 0:256],
                x_t[:, 2 * dp:2 * dp + 2, xh, stl],
                wv_t[:, 2 * dp:2 * dp + 2, wh, :],
                start=(i == 0), stop=(i == 11), perf_mode=DR)
        if half == 1:
            nc.scalar.copy(
                v_t[st][:].rearrange("p (h f) -> p h f", h=4)[:, :, 0:64],
                pv[:, 0:256].rearrange("p (h f) -> p h f", h=4))
            nc.gpsimd.memset(
                v_t[st][:].rearrange("p (h f) -> p h f", h=4)[:, :, 64:65],
                1.0)

    for st in sts:
        for half in range(2):
            yield (lambda st=st, half=half: v_piece(st, half))


def _proj_chunk_pieces(nc, sc_i, pools, x_t, wqk_t, wv_t, cos_t, sin_t,
                       qc, rstage, v_t, ropet, include_v=True):
    """Yield closures emitting projection chunk sc_i piecewise (~2 matmuls
    per piece) so it can be used as PE filler inside attention loops.
    pools: (qk_alloc, v_alloc) -> psum AP factories."""
    qk_alloc, v_alloc = pools
    sl = slice(sc_i * 512, (sc_i + 1) * 512)
    ps = {}

    # 3 hi/lo terms x 4 d-pairs of fp8 DoubleRow matmuls, 2 per piece.
    # terms: (hi W x hi x), (hi W x lo x), (lo W x hi x)
    TERMS = ((0, 0), (0, 1), (1, 0))

    def qk_piece(e, pc):
        if pc == 0:
            ps[e] = qk_alloc(e)
        p = ps[e]
        for i in (2 * pc, 2 * pc + 1):
            (wh, xh), dp = TERMS[i // 4], i % 4
            nc.tensor.matmul(
                p[:, 0:512],
                wqk_t[:, 2 * dp:2 * dp + 2, wh, e * 128:(e + 1) * 128],
                x_t[:, 2 * dp:2 * dp + 2, xh, sl],
                start=(i == 0), stop=(i == 11), perf_mode=DR)
        if pc == 5 and e % 2 == 1:
            _emit_rope(nc, sc_i, sl, e, ps, rstage, cos_t, sin_t, qc, ropet)

    # chunk 3 runs K blocks first: the last rope then covers only Q3,
    # whose consumer (quarter A3) runs second - the first quarter's psum
    # WAR on these banks clears sooner
    e_order = (2, 3, 0, 1) if sc_i == 3 else (0, 1, 2, 3)
    for e in e_order:
        for pc in range(6):
            yield (lambda e=e, pc=pc: qk_piece(e, pc))
    if include_v:
        yield from _v_pieces(nc, range(4 * sc_i, 4 * (sc_i + 1)), v_alloc,
                             x_t, wv_t, v_t)


def _emit_rope(nc, sc_i, sl, e, ps, rstage, cos_t, sin_t, qc, ropet):
    """RoPE for the (e-1, e) block pair, writing into the persistent
    rstage[e] tiles (block layout, full S).  Scatter DMAs into qc/kc are
    batched separately (see _scatter) - except Q of chunk 3, which quarter
    A3 needs immediately after phase A."""
    A, Bp = ps[e - 1], ps[e]
    oA = rstage[e - 1][:, sl]
    oB = rstage[e][:, sl]
    t1 = ropet.tile([128, 512], F32, tag="t1")
    t2 = ropet.tile([128, 512], F32, tag="t2")
    t3 = ropet.tile([128, 512], F32, tag="t3")
    t4 = ropet.tile([128, 512], F32, tag="t4")
    # oA = A*cos - B*sin ; oB = B*cos + A*sin
    # products on DVE (PSUM reads); combines on GpSimd (SBUF-only)
    nc.vector.tensor_tensor(t1[:], A[:, 0:512], cos_t[:, sl],
                            mybir.AluOpType.mult)
    nc.vector.tensor_tensor(t2[:], Bp[:, 0:512], sin_t[:, sl],
                            mybir.AluOpType.mult)
    nc.gpsimd.tensor_tensor(oA, t1[:], t2[:], mybir.AluOpType.subtract)
    nc.vector.tensor_tensor(t3[:], Bp[:, 0:512], cos_t[:, sl],
                            mybir.AluOpType.mult)
    nc.vector.tensor_tensor(t4[:], A[:, 0:512], sin_t[:, sl],
                            mybir.AluOpType.mult)
    nc.gpsimd.tensor_tensor(oB, t3[:], t4[:], mybir.AluOpType.add)
    if sc_i == 0 and e == 1:
        _scatter(nc, rstage, 0, qc, 0, 512)
    if sc_i == 3 and e == 1:
        _scatter(nc, rstage, 0, qc, 1536, 2048)


def _scatter(nc, rstage, base_e, dsts, c0, c1):
    """Scatter the [c0:c1] column span of rstage blocks (base_e, base_e+1)
    into head-contiguous layout: head h x1 -> dsts[h//2][64*(h%2):+32],
    x2 -> +32:+64."""
    for h in range(HPG):
        dt_ = dsts[h // 2]
        po = 64 * (h % 2)
        nc.sync.dma_start(dt_[po:po + 32, c0:c1],
                          rstage[base_e][32 * h:32 * h + 32, c0:c1])
        nc.sync.dma_start(dt_[po + 32:po + 64, c0:c1],
                          rstage[base_e + 1][32 * h:32 * h + 32, c0:c1])


def _attention_zip2(nc, attps, ptp, qlo, qhi, xaccs, kc, qc, v_t,
                    maskT, ident):
    """Both head-pairs of a (small) quarter interleaved tile-by-tile: no
    hp-seam pipeline refill, ACT stays fed.  Used for quarter 0 only (its
    filler/ops pool is free to hold the second pair's accumulators)."""
    t_end = qhi // 128
    pend = {0: None, 1: None}
    for t in range(t_end):
        ktl = slice(t * 128, (t + 1) * 128)
        off = max(qlo, 128 * t)
        n = qhi - off
        base = off % 512
        diag = off == 128 * t
        for hp in (0, 1):
            heads = (2 * hp, 2 * hp + 1)
            sc = attps.tile([128, 1024], F32, tag="sc",
                            name=f"scz_{t}_{hp}")
            for j, h in enumerate(heads):
                hs = slice(64 * j, 64 * j + 64)
                nc.tensor.matmul(
                    sc[:, 512 * j + base:512 * j + base + n], kc[hp][hs, ktl],
                    qc[hp][hs, off:off + n], start=True, stop=not diag)
                if diag:
                    nc.tensor.matmul(
                        sc[:, 512 * j + base:512 * j + base + 128],
                        maskT[:], ident[:], start=False, stop=True)
            pt = ptp.tile([128, 1024], BF16, tag="pt", name=f"ptz_{t}_{hp}")
            scv = sc.rearrange("p (j c) -> p j c", j=2)
            ptv = pt.rearrange("p (j c) -> p j c", j=2)
            nc.scalar.activation(
                ptv[:, :, base:base + n], scv[:, :, base:base + n],
                mybir.ActivationFunctionType.Exp, scale=SCALE / QK_SCALE)
            if pend[hp] is not None:
                _emit_pv(nc, heads, xaccs, v_t, qlo, *pend[hp], t_end)
            pend[hp] = (pt, base, n, off, t)
    for hp in (0, 1):
        _emit_pv(nc, (2 * hp, 2 * hp + 1), xaccs, v_t, qlo, *pend[hp], t_end)


def _prestart_t0(nc, attps, ptp, qh, hp, kc, qc, maskT, ident, stash):
    """Emit the first score tile + exp of pair (qh, hp) early (as a filler
    in the previous pair's endgame) so the ACT pipeline stays warm across
    the seam; _attention_pair picks it up via `stash`."""
    qlo = 512 * qh
    off, n, base = qlo, 512, 0
    diag = qlo == 0
    heads = (2 * hp, 2 * hp + 1)
    sc = attps.tile([128, 1024], F32, tag="sc", name=f"scp_{qh}_{hp}")
    for j, h in enumerate(heads):
        hs = slice(64 * j, 64 * j + 64)
        nc.tensor.matmul(
            sc[:, 512 * j:512 * j + n], kc[hp][hs, 0:128],
            qc[hp][hs, off:off + n], start=True, stop=not diag)
        if diag:
            nc.tensor.matmul(sc[:, 512 * j:512 * j + 128],
                             maskT[:], ident[:], start=False, stop=True)
    pt = ptp.tile([128, 1024], BF16, tag="pt", name=f"ptp_{qh}_{hp}")
    scv = sc.rearrange("p (j c) -> p j c", j=2)
    ptv = pt.rearrange("p (j c) -> p j c", j=2)
    nc.scalar.activation(
        ptv[:, :, base:base + n], scv[:, :, base:base + n],
        mybir.ActivationFunctionType.Exp, scale=SCALE / QK_SCALE)
    stash[(qh, hp)] = (pt, base, n, off, 0)


def _attention_pair(nc, attps, ptp, heads, qlo, qhi, xaccs, kc, qc, v_t,
                    maskT, ident, sched, pend0=None):
    """QK->exp->PV for a pair of heads over q range [qlo, qhi).
    Both heads' scores for one k-tile land in a single 2-bank PSUM tile so
    ONE exp instruction covers them.  Causal masking of diagonal blocks:
    a DVE add of a strict-lower-triangular -1e8 constant onto the diag
    region of the score psum (k > q), so exp underflows those entries to
    exactly 0.  Software-pipelined: PV(t-1) is emitted
    after scores(t)+filler so the PE never waits on exp(t-1) latency."""
    t_end = qhi // 128
    hp = heads[0] // 2
    pend = pend0
    for piece in sched.get(-1, ()):
        piece()
    if pend0 is not None:
        for piece in sched.get(0, ()):
            piece()
    for t in range(1 if pend0 is not None else 0, t_end):
        ktl = slice(t * 128, (t + 1) * 128)
        off = max(qlo, 128 * t)
        n = qhi - off
        base = off % 512
        diag = off == 128 * t
        sc = attps.tile([128, 1024], F32, tag="sc")
        for j, h in enumerate(heads):
            hs = slice(64 * j, 64 * j + 64)
            nc.tensor.matmul(
                sc[:, 512 * j + base:512 * j + base + n], kc[hp][hs, ktl],
                qc[hp][hs, off:off + n], start=True, stop=not diag)
            if diag:
                nc.tensor.matmul(
                    sc[:, 512 * j + base:512 * j + base + 128],
                    maskT[:], ident[:], start=False, stop=True)
        pt = ptp.tile([128, 1024], BF16, tag="pt")
        scv = sc.rearrange("p (j c) -> p j c", j=2)
        ptv = pt.rearrange("p (j c) -> p j c", j=2)
        nc.scalar.activation(
            ptv[:, :, base:base + n], scv[:, :, base:base + n],
            mybir.ActivationFunctionType.Exp, scale=SCALE / QK_SCALE)
        for piece in sched.get(t, ()):
            piece()
        if pend is not None:
            _emit_pv(nc, heads, xaccs, v_t, qlo, *pend, t_end)
        pend = (pt, base, n, off, t)
    _emit_pv(nc, heads, xaccs, v_t, qlo, *pend, t_end)


def _emit_pv(nc, heads, xaccs, v_t, qlo, pt, base, n, off, t, t_end):
    for j, h in enumerate(heads):
        nc.tensor.matmul(
            xaccs[h][:, off - qlo:off - qlo + n],
            v_t[t][:, 65 * h:65 * h + 65],
            pt[:, 512 * j + base:512 * j + base + n],
            start=(t == 0), stop=(t == t_end - 1))


def _body(nc, tc, xT, wqkT, wvT, cos4, sin4, woT, out):
    with tc.tile_pool(name="const", bufs=1) as constp, \
         tc.tile_pool(name="rot", bufs=1) as rotp, \
         tc.tile_pool(name="vsd", bufs=1) as vsd, \
         tc.tile_pool(name="xnorm", bufs=1) as xnp, \
         tc.tile_pool(name="wo", bufs=1) as wop, \
         tc.tile_pool(name="xw", bufs=1) as xw, \
         tc.tile_pool(name="ptp", bufs=5) as ptp, \
         tc.tile_pool(name="nrm", bufs=3) as nrmp, \
         tc.tile_pool(name="oout", bufs=4) as ooutp:
        # maskT/ident implement in-matmul causal masking of diagonal
        # score blocks (adds -1e8 where k > q before the exp)
        maskT = constp.tile([128, 128], BF16)
        make_upper_triangular(nc, maskT[:], val=-1.0e8, diag=False)
        ident = constp.tile([128, 128], BF16)
        make_identity(nc, ident[:])
        ones64 = constp.tile([1, 64], F32)
        nc.vector.memset(ones64[:], 1.0)
        ones64b = constp.tile([1, 64], BF16)
        nc.vector.memset(ones64b[:], 1.0)
        # dummy exp: pulls the ACT table load into phase A (off the first
        # attention quarter's critical path)
        dummy = constp.tile([1, 16], F32)
        nc.scalar.activation(dummy[:], ones64[:, 0:16],
                             mybir.ActivationFunctionType.Exp, scale=1.0)
        # head-contiguous rotated Q/K: qc[i] holds heads 2i, 2i+1 with each
        # head's 64 features (x1;x2) contiguous on partitions
        qc = [rotp.tile([128, S], BF16, name=f"qc_{i}") for i in range(2)]
        kc = [rotp.tile([128, S], BF16, name=f"kc_{i}") for i in range(2)]
        # persistent RoPE output staging (block layout: QA QB KA KB)
        rstage = [rotp.tile([128, S], BF16, name=f"rs_{e}") for e in range(4)]
        v_t = [vsd.tile([128, 260], BF16, name=f"v_{st}") for st in range(ST)]
        xn = [xnp.tile([128, S], BF16, name=f"xn_{hp}") for hp in range(2)]
        wo_t = [wop.tile([128, D], BF16, name=f"wo_{d2}") for d2 in range(2)]

        # d-chunked fp8 hi/lo operands live in single big tiles (few, large
        # DMAs: the hwdge descriptor unit costs ~0.6us per DMA instruction)
        x_t = xw.tile([128, DT, 2, S], F8, name="xbig")
        wqk_t = xw.tile([128, DT, 2, 512], F8, name="wqkbig")
        wv_t = xw.tile([128, DT, 2, 256], F8, name="wvbig")
        cos_t = xw.tile([128, S], F32)
        sin_t = xw.tile([128, S], F32)
        xTr = xT.rearrange("(d p) j c -> p d j c", d=DT)
        wqkr = wqkT.rearrange("(d p) j c -> p d j c", d=DT)
        wvr = wvT.rearrange("(d p) j c -> p d j c", d=DT)
        # loads in consumption order: wqk halves zipped with x first-halves
        nc.sync.dma_start(wqk_t[:, 0:4], wqkr[:, 0:4])
        nc.sync.dma_start(x_t[:, 0:2, 0, 0:1024], xTr[:, 0:2, 0, 0:1024])
        nc.sync.dma_start(x_t[:, 0:2, 1, 0:1024], xTr[:, 0:2, 1, 0:1024])
        nc.sync.dma_start(wqk_t[:, 4:8], wqkr[:, 4:8])
        for dp in range(1, 4):
            nc.sync.dma_start(x_t[:, 2 * dp:2 * dp + 2, 0, 0:1024],
                              xTr[:, 2 * dp:2 * dp + 2, 0, 0:1024])
            nc.sync.dma_start(x_t[:, 2 * dp:2 * dp + 2, 1, 0:1024],
                              xTr[:, 2 * dp:2 * dp + 2, 1, 0:1024])
        nc.sync.dma_start(cos_t[:], cos4[:])
        nc.sync.dma_start(sin_t[:], sin4[:])
        nc.sync.dma_start(wv_t[:], wvr[:])
        for dp in range(4):
            for j in range(2):
                nc.sync.dma_start(x_t[:, 2 * dp:2 * dp + 2, j, 1024:2048],
                                  xTr[:, 2 * dp:2 * dp + 2, j, 1024:2048])
        for d2 in range(2):
            nc.sync.dma_start(wo_t[d2][:], woT[d2 * 128:(d2 + 1) * 128, :])

        # ============ Phase A: projections (all 4 chunks) ============
        with tc.tile_pool(name="ropet", bufs=4) as ropet:
            with tc.tile_pool(name="qkps", bufs=3, space="PSUM") as qkps, \
                 tc.tile_pool(name="vps", bufs=2, space="PSUM") as vps:
                for sc_i in range(4):
                    pools = (
                        lambda e, s=sc_i: qkps.tile(
                            [128, 512], F32, tag=f"qk{e % 2}",
                            name=f"qk{e}_{s}"),
                        lambda st: vps.tile([128, 256], F32, tag="vps",
                                            name=f"pv_{st}"),
                    )
                    # V of chunk 3 is deferred into the A3 attention loop
                    # (its PVs only need v_t[12..15] near the t-loop end).
                    for piece in _proj_chunk_pieces(
                            nc, sc_i, pools, x_t, wqk_t, wv_t, cos_t, sin_t,
                            qc, rstage, v_t, ropet, include_v=(sc_i < 3)):
                        piece()
                    if sc_i == 1:
                        # A3's first 8 k-tiles only need kc columns 0:1024
                        _scatter(nc, rstage, 2, kc, 0, 1024)
                    if sc_i == 3:
                        _scatter(nc, rstage, 2, kc, 1024, 2048)
                        _scatter(nc, rstage, 0, qc, 512, 1536)

            # ======== Phase B: attention quarters (3,2,1,0) + out-proj ====
            # Descending order puts the smallest quarter last (short tail);
            # the out-projection of each processed quarter becomes PE filler
            # work inside the next quarter's attention loop.
            # psum: sc [128,1024]x2 + xacc [65,512]x3 + ops [128,512]x1 = 8.
            # xacc is 3-deep so a new head-pair's accumulators never wait on
            # the previous pair's norm reads (the boundary serializer).
            with tc.tile_pool(name="attps", bufs=2, space="PSUM") as attps, \
                 tc.tile_pool(name="xaccps", bufs=2, space="PSUM") as xaccps, \
                 tc.tile_pool(name="ops", bufs=2, space="PSUM") as opsp:
                ops_alloc = (lambda st: opsp.tile([128, 512], F32, tag="ops",
                                                  name=f"dpv_{st}"))
                seq = [(0, 0), (0, 1), (3, 0), (3, 1), (2, 0), (2, 1),
                       (1, 0), (1, 1)]
                prestash = {}
                prev_qh = None
                for qh in (0, 3, 2, 1):
                    qlo, qhi = 512 * qh, 512 * (qh + 1)
                    t_end = 4 * (qh + 1)
                    # Per-hp filler schedules (tile -> pieces): deferred
                    # V-projections are EAGER (one per tile from t=0, their
                    # PVs consume them later in the same loop); out-proj
                    # units of the previous quarter are spread evenly.
                    sched = [{}, {}]
                    if qh == 3:
                        # all 8 pieces in hp0, positioned as late as each
                        # PV dependency allows (the diag region is where ACT
                        # outpaces PE and needs PE filler)
                        vp = list(_v_pieces(nc, range(12, 16), ops_alloc,
                                            x_t, wv_t, v_t))
                        vpos = [0, 8, 9, 10, 11, 12, 13, 14]
                        for i, p in enumerate(vp):
                            sched[0].setdefault(vpos[i], []).append(p)
                    if prev_qh is not None:
                        ou = list(_outproj_units(nc, opsp, ooutp, xn, wo_t,
                                                 out, prev_qh,
                                                 act_c1=False))
                        # one unit at the hp-boundary warmup, the rest in the
                        # trailing diag region where ACT outpaces PE
                        pos = [t_end - 4, t_end - 3, t_end - 2, t_end - 1]
                        for k in range(2):
                            for i, p in enumerate(ou[4 * k:4 * k + 4]):
                                sched[k].setdefault(pos[i], []).append(p)

                    for hp in range(2):
                        # last filler slot: prestart the NEXT pair's first
                        # score tile + exp so ACT never idles across seams
                        i = seq.index((qh, hp))
                        if i + 1 < len(seq):
                            nqh, nhp = seq[i + 1]
                            sched[hp].setdefault(t_end - 1, []).append(
                                lambda nqh=nqh, nhp=nhp: _prestart_t0(
                                    nc, attps, ptp, nqh, nhp, kc, qc,
                                    maskT, ident, prestash))
                        heads = (2 * hp, 2 * hp + 1)
                        xaccs = {}
                        for h in heads:
                            xaccs[h] = xaccps.tile([65, 512], F32, tag="xacc",
                                                   name=f"xacc_{qh}_{h}")
                        _attention_pair(nc, attps, ptp, heads, qlo, qhi,
                                        xaccs, kc, qc, v_t, maskT, ident,
                                        sched[hp],
                                        pend0=prestash.pop((qh, hp), None))
                        if qh == 1 and hp == 1:
                            # pre-start the tail's d2=0 accumulations (they
                            # only need xn[0]) so the PE overlaps the final
                            # norm ladder instead of waiting behind it
                            tail_p1, tail_p2 = _tail_outproj(
                                nc, attps, opsp, ooutp, xn, wo_t, out)
                            tail_p1()
                        for h in heads:
                            xacc = xaccs[h]
                            # stash the accumulators to SBUF on ACT first:
                            # this releases the xacc PSUM bank in ~0.6us so
                            # the next pair's PVs never wait on the norm
                            # ladder (which otherwise holds the bank ~5us -
                            # reciprocal on a [1,512] single-partition AP is
                            # ~3.3us on DVE).  The ladder then runs from
                            # SBUF fully off the PE critical path.
                            xs = nrmp.tile([65, 512], F32, tag="xs")
                            # copy on DVE: ACT (exp) is the busier engine
                            # in the attention steady state
                            nc.vector.tensor_scalar_mul(xs[:], xacc[:], 1.0)
                            xsrc = xs
                            # den must sit at partition base 0: the
                            # custom-DVE reciprocal_approx_fast corrupts
                            # data when its input AP has a non-zero base
                            # partition (verified on hw), so stage row 64
                            # down with a cheap DVE copy first.
                            den = nrmp.tile([1, 512], F32, tag="den")
                            nc.vector.tensor_scalar_mul(den[:], xsrc[64:65, :],
                                                        1.0)
                            recip = nrmp.tile([1, 512], F32, tag="recip")
                            nc.vector.reciprocal_approx_fast(recip[:], den[:])
                            rb = nrmp.tile([64, 512], F32, tag="rb")
                            nc.gpsimd.partition_broadcast(rb[:], recip[:])
                            dst = xn[h // 2][64 * (h % 2):64 * (h % 2) + 64, :]
                            nc.vector.tensor_tensor(
                                dst[:, qlo:qhi], xsrc[0:64, :], rb[:],
                                mybir.AluOpType.mult)
                    prev_qh = qh
                # tail: rest of the last quarter's out-projection
                tail_p2()


def _to_bf16(a):
    import ml_dtypes
    return np.ascontiguousarray(a.astype(ml_dtypes.bfloat16))


def _to_hilo8(a, scale):
    """[R, C] f32 -> [R, 2, C] fp8 e4m3 (hi, residual-lo) after scaling."""
    import ml_dtypes
    E4 = ml_dtypes.float8_e4m3
    a = np.asarray(a, np.float32) * scale
    hi = a.astype(E4)
    lo = (a - hi.astype(np.float32)).astype(E4)
    return np.ascontiguousarray(np.stack([hi, lo], axis=1))


def prepare_in_maps(inputs, positions, W_in, W_out):
    """Build per-core input shards (all host-side numpy prep)."""
    inputs = np.ascontiguousarray(inputs, dtype=np.float32)
    W_in = np.ascontiguousarray(W_in, dtype=np.float32)
    W_out = np.ascontiguousarray(W_out, dtype=np.float32)
    positions = np.asarray(positions)

    inv_freq = 1.0 / (ROPE_BASE ** (np.arange(0, HD, 2, dtype=np.float32) / HD))
    in_maps = []
    for core in range(N_CORES):
        b, g = divmod(core, G)
        heads = [g * HPG + h for h in range(HPG)]

        xTb = inputs[b].T                                          # [D, S]

        # RoPE tables [128, S]: rows = 4 copies of the 32 freqs
        ang = positions[b].astype(np.float32)[None, :] * inv_freq[:, None]
        cos4 = np.tile(np.cos(ang), (4, 1)).astype(np.float32)
        sin4 = np.tile(np.sin(ang), (4, 1)).astype(np.float32)

        # QK weight blocks: QA/QB/KA/KB, each 128 rows (4 heads x 32)
        def rows(base_off):
            idx = []
            for h in heads:
                idx.extend(h * 3 * HD + base_off + f for f in range(32))
            return idx
        qk_idx = rows(0) + rows(32) + rows(64) + rows(96)
        wqkT = W_in[qk_idx].T                                      # [D, 512]

        v_idx = []
        for h in heads:
            v_idx.extend(h * 3 * HD + 2 * HD + f for f in range(HD))
        wvT = W_in[v_idx].T                                        # [D, 256]

        # W_out columns for this group's feature slice, transposed
        dsl = [h * HD + f for h in heads for f in range(HD)]
        woT = W_out[:, dsl].T                                      # [256, D]

        in_maps.append({
            "xT": _to_hilo8(xTb, X_SCALE),
            "wqkT": _to_hilo8(wqkT, W_SCALE),
            "wvT": _to_hilo8(wvT, W_SCALE),
            "cos4": cos4, "sin4": sin4, "woT": _to_bf16(woT),
        })
    return in_maps


def assemble_output(results):
    """Sum the 4 per-group partials (bf16) for each batch."""
    out = np.zeros((B, S, D), dtype=np.float32)
    for core in range(N_CORES):
        b = core // G
        out[b] += np.asarray(results[core]["out"], dtype=np.float32)
    return out


_NC_CACHE = {}


def get_nc():
    if "nc" not in _NC_CACHE:
        _NC_CACHE["nc"] = build_nc()
    return _NC_CACHE["nc"]


def kernel(inputs, positions, W_in, W_out):
    nc = get_nc()
    in_maps = prepare_in_maps(inputs, positions, W_in, W_out)
    res = bass_utils.run_bass_kernel_spmd(
        nc, in_maps, core_ids=list(range(N_CORES)))
    return assemble_output(res.results)



# revision 19
# speedup vs baseline: 1.0194x; 1.0194x over previous
"""Trainium2 Bass kernel for nn_AttentionBlock (B=2, S=2048, D=1024, H=16, HD=64).

Sharding: 8 cores = 2 batches x 4 head-groups (4 heads each).
Each core computes, for its (batch b, head-group g):
  - fused QK projection (RoPE'd, feature-transposed layout) + V projection
  - causal attention for its 4 heads (scores computed transposed, softmax
    denominator via an appended ones-column in the PV matmul)
  - a PARTIAL output projection: x_g @ W_out[:, d-slice].T  -> [S, D] partial
The host sums the 4 partials per batch (linear unshard step) - no on-device
collectives needed.

Schedule: phase A runs the QKV projections for all four 512-column chunks
(V of chunk 3 deferred), then attention quarters are processed in order
(0, 3, 2, 1).  Within a quarter the t-loop is software-pipelined
(scores(t) -> exp(t) -> PV(t-1)); both heads of a pair share one 2-bank
PSUM score tile so a single exp instruction covers them; causal masking is
done inside the scores matmul group (a -1e8 upper-triangular add) so no
vector-engine op sits on the critical chain.  The out-projection of each
finished quarter and the deferred V tiles are emitted as PE "filler"
pieces inside the next quarter's loop, positioned in the trailing diagonal
region where the ACT engine otherwise outpaces the PE; each pair's final
filler slot also pre-starts the NEXT pair's first score tile + exp so the
ACT pipeline stays warm across seams.
PSUM: projections 6+2 banks; attention 4 (scores) + 2 (xacc) + 2 (outproj).

Dtypes: the QKV projections run as fp8(e4m3) DoubleRow matmuls on hi/lo
split operands (x*8, W_in*32: Wh(xh+xl) + Wl xh), which both doubles PE
throughput and carries ~11 effective mantissa bits; everything downstream
(RoPE output, scores, pt, v, xn, W_out) is bf16 with fp32 PSUM
accumulation; output partials are written bf16 (rescaled by 1/256).

Self-contained: hardcodes all shapes; imports only concourse + numpy.
"""
import math

import numpy as np

import concourse.bass as bass  # noqa: F401
import concourse.bacc as bacc
import concourse.mybir as mybir
import concourse.tile as tile
from concourse import bass_utils
from concourse.masks import make_identity, make_upper_triangular

B, S, D, H = 2, 2048, 1024, 16
HD = D // H            # 64
G = 4                  # head-groups (cores per batch)
HPG = H // G           # 4 heads per group
N_CORES = 8
ROPE_BASE = 10000.0
F32 = mybir.dt.float32
F32R = mybir.dt.float32r
BF16 = mybir.dt.bfloat16
F8 = mybir.dt.float8e4
DR = mybir.MatmulPerfMode.DoubleRow

KT = S // 128          # 16 k-tiles of 128
ST = S // 128          # 16 s-tiles
DT = D // 128          # 8 d-chunks
X_SCALE = 8.0          # fp8 pre-scale on x
W_SCALE = 32.0         # fp8 pre-scale on W_in (lifts W out of e4m3 subnormals)
QK_SCALE = (X_SCALE * W_SCALE) ** 2   # scores carry (x*W)^2 scaling
V_SCALE = X_SCALE * W_SCALE           # v / xn / out-proj psum carry this
SCALE = 1.0 / math.sqrt(HD)


def build_nc():
    nc = bacc.Bacc("TRN2", target_bir_lowering=False, debug=False,
                   num_devices=N_CORES)

    # x (x8) and projection weights (x32) in fp8 e4m3 hi/lo pairs: the
    # projections run as DoubleRow fp8 matmuls (Wh(xh+xl) + Wl xh), which
    # stream two contraction rows per cycle on the PE.
    xT = nc.dram_tensor("xT", [D, 2, S], F8, kind="ExternalInput").ap()
    # QK weights, transposed+permuted: columns = [QA|QB|KA|KB] of 128 each.
    wqkT = nc.dram_tensor("wqkT", [D, 2, 512], F8, kind="ExternalInput").ap()
    # V weights, transposed: columns = 4 heads x 64 feats.
    wvT = nc.dram_tensor("wvT", [D, 2, 256], F8, kind="ExternalInput").ap()
    # RoPE tables, [128, S]: rows = 4x (32 freqs).
    cos4 = nc.dram_tensor("cos4", [128, S], F32, kind="ExternalInput").ap()
    sin4 = nc.dram_tensor("sin4", [128, S], F32, kind="ExternalInput").ap()
    # W_out columns for this group's features, transposed: [256, D].
    woT = nc.dram_tensor("woT", [256, D], BF16, kind="ExternalInput").ap()
    out = nc.dram_tensor("out", [S, D], BF16, kind="ExternalOutput").ap()

    with tile.TileContext(nc) as tc:
        _body(nc, tc, xT, wqkT, wvT, cos4, sin4, woT, out)
    nc.compile()
    return nc


def _outproj_units(nc, opsp, ooutp, xn, wo_t, out, qh, tail=False,
                   act_c1=False):
    """Yield closures, each emitting one (st, c) out-projection piece
    (2 matmuls + psum->sbuf copy + dma on the second half).  Used as PE
    filler work.  The tail variant allocates 2-bank psum tiles (from the
    then-idle attention score pool) so both halves run concurrently, and
    puts half the copies on ACT so the final drain parallelizes."""
    state = {}
    pstate = {}

    def unit(st, c):
        stl = slice(st * 128, (st + 1) * 128)
        if st not in state:
            state[st] = ooutp.tile([128, 1024], BF16, tag="ot",
                                   name=f"ot_{qh}_{st}")
        ot = state[st]
        if tail:
            if st not in pstate:
                pstate[st] = opsp.tile([128, 1024], F32, tag="sc",
                                       name=f"pot_{qh}_{st}")
            po = pstate[st][:, c * 512:(c + 1) * 512]
        else:
            po = opsp.tile([128, 512], F32, tag="ops",
                           name=f"po_{qh}_{st}_{c}")
        for d2 in range(2):
            nc.tensor.matmul(
                po, xn[d2][:, stl], wo_t[d2][:, c * 512:(c + 1) * 512],
                start=(d2 == 0), stop=(d2 == 1))
        if c == 1 and (tail or act_c1):
            # tail copies on ACT so the final drain parallelizes with DVE
            nc.scalar.mul(ot[:, c * 512:(c + 1) * 512], po, 1.0 / V_SCALE)
        else:
            nc.vector.tensor_scalar_mul(ot[:, c * 512:(c + 1) * 512], po,
                                        1.0 / V_SCALE)
        nc.sync.dma_start(out[stl, c * 512:(c + 1) * 512],
                          ot[:, c * 512:(c + 1) * 512])

    cs = (1, 0) if tail else (0, 1)
    for st in range(4 * qh, 4 * qh + 4):
        for c in cs:
            yield (lambda st=st, c=c: unit(st, c))


def _tail_outproj(nc, attps, opsp, ooutp, xn, wo_t, out):
    """Two-part tail for quarter 1 (st 4..7): part1 emits the d2=0 halves of
    st4/st5/st6 (xn[0]-only dependency, runnable during the final norm
    ladder); part2 closes them and runs st7, copies split ACT(c1)/DVE(c0).
    st6 borrows the (tail-idle) ops pool so it doesn't wait on st4/st5's
    sc-slot copies; st7 reuses st4's sc slot.  Each finished half DMAs
    immediately (transfer latency beats issue-slot count at the drain)."""
    state = {}
    pstate = {}

    def ensure(st):
        if st not in state:
            state[st] = ooutp.tile([128, 1024], BF16, tag="ot",
                                   name=f"ot_t_{st}")
            if st == 6:
                pstate[st] = [opsp.tile([128, 512], F32, tag="ops",
                                        name=f"pot_t_{st}_{c}")
                              for c in range(2)]
            else:
                pt_ = attps.tile([128, 1024], F32, tag="sc",
                                 name=f"pot_t_{st}")
                pstate[st] = [pt_[:, 0:512], pt_[:, 512:1024]]

    def mm(st, c, d2, start, stop):
        stl = slice(st * 128, (st + 1) * 128)
        po = pstate[st][c]
        nc.tensor.matmul(po, xn[d2][:, stl],
                         wo_t[d2][:, c * 512:(c + 1) * 512],
                         start=start, stop=stop)

    def finish(st, c):
        stl = slice(st * 128, (st + 1) * 128)
        po = pstate[st][c]
        ot = state[st]
        if c == 1:
            nc.scalar.mul(ot[:, c * 512:(c + 1) * 512], po, 1.0 / V_SCALE)
        else:
            nc.vector.tensor_scalar_mul(ot[:, c * 512:(c + 1) * 512], po,
                                        1.0 / V_SCALE)
        nc.sync.dma_start(out[stl, c * 512:(c + 1) * 512],
                          ot[:, c * 512:(c + 1) * 512])

    def part1():
        for st in (4, 5, 6):
            ensure(st)
            for c in (0, 1):
                mm(st, c, 0, True, False)

    def part2():
        for st in (4, 5):
            for c in (1, 0):
                mm(st, c, 1, False, True)
                finish(st, c)
        for c in (1, 0):
            mm(6, c, 1, False, True)
            finish(6, c)
        ensure(7)
        for c in (1, 0):
            mm(7, c, 0, True, False)
            mm(7, c, 1, False, True)
            finish(7, c)

    return part1, part2


def _v_pieces(nc, sts, v_alloc, x_t, wv_t, v_t):
    """Yield closures emitting the V-projection of s-tiles `sts`, two pieces
    (6 fp8 DoubleRow matmuls) each."""
    ps = {}
    TERMS = ((0, 0), (0, 1), (1, 0))  # (W hi/lo, x hi/lo)

    def v_piece(st, half):
        if half == 0:
            ps[st] = v_alloc(st)
        pv = ps[st]
        stl = slice(st * 128, (st + 1) * 128)
        for i in range(6 * half, 6 * half + 6):
            (wh, xh), dp = TERMS[i // 4], i % 4
            nc.tensor.matmul(
                pv[:, 0:256],
                x_t[:, 2 * dp:2 * dp + 2, xh, stl],
                wv_t[:, 2 * dp:2 * dp + 2, wh, :],
                start=(i == 0), stop=(i == 11), perf_mode=DR)
        if half == 1:
            nc.scalar.copy(
                v_t[st][:].rearrange("p (h f) -> p h f", h=4)[:, :, 0:64],
                pv[:, 0:256].rearrange("p (h f) -> p h f", h=4))
            nc.gpsimd.memset(
                v_t[st][:].rearrange("p (h f) -> p h f", h=4)[:, :, 64:65],
                1.0)

    for st in sts:
        for half in range(2):
            yield (lambda st=st, half=half: v_piece(st, half))


def _proj_chunk_pieces(nc, sc_i, pools, x_t, wqk_t, wv_t, cos_t, sin_t,
                       qc, rstage, v_t, ropet, include_v=True):
    """Yield closures emitting projection chunk sc_i piecewise (~2 matmuls
    per piece) so it can be used as PE filler inside attention loops.
    pools: (qk_alloc, v_alloc) -> psum AP factories."""
    qk_alloc, v_alloc = pools
    sl = slice(sc_i * 512, (sc_i + 1) * 512)
    ps = {}

    # 3 hi/lo terms x 4 d-pairs of fp8 DoubleRow matmuls, 2 per piece.
    # terms: (hi W x hi x), (hi W x lo x), (lo W x hi x)
    TERMS = ((0, 0), (0, 1), (1, 0))

    def qk_piece(e, pc):
        if pc == 0:
            ps[e] = qk_alloc(e)
        p = ps[e]
        for i in (2 * pc, 2 * pc + 1):
            (wh, xh), dp = TERMS[i // 4], i % 4
            nc.tensor.matmul(
                p[:, 0:512],
                wqk_t[:, 2 * dp:2 * dp + 2, wh, e * 128:(e + 1) * 128],
                x_t[:, 2 * dp:2 * dp + 2, xh, sl],
                start=(i == 0), stop=(i == 11), perf_mode=DR)
        if pc == 5 and e % 2 == 1:
            _emit_rope(nc, sc_i, sl, e, ps, rstage, cos_t, sin_t, qc, ropet)

    # chunk 3 runs K blocks first: the last rope then covers only Q3,
    # whose consumer (quarter A3) runs second - the first quarter's psum
    # WAR on these banks clears sooner
    e_order = (2, 3, 0, 1) if sc_i == 3 else (0, 1, 2, 3)
    for e in e_order:
        for pc in range(6):
            yield (lambda e=e, pc=pc: qk_piece(e, pc))
    if include_v:
        yield from _v_pieces(nc, range(4 * sc_i, 4 * (sc_i + 1)), v_alloc,
                             x_t, wv_t, v_t)


def _emit_rope(nc, sc_i, sl, e, ps, rstage, cos_t, sin_t, qc, ropet):
    """RoPE for the (e-1, e) block pair, writing into the persistent
    rstage[e] tiles (block layout, full S).  Scatter DMAs into qc/kc are
    batched separately (see _scatter) - except Q of chunk 3, which quarter
    A3 needs immediately after phase A."""
    A, Bp = ps[e - 1], ps[e]
    oA = rstage[e - 1][:, sl]
    oB = rstage[e][:, sl]
    t1 = ropet.tile([128, 512], F32, tag="t1")
    t2 = ropet.tile([128, 512], F32, tag="t2")
    t3 = ropet.tile([128, 512], F32, tag="t3")
    t4 = ropet.tile([128, 512], F32, tag="t4")
    # oA = A*cos - B*sin ; oB = B*cos + A*sin
    # products on DVE (PSUM reads); combines on GpSimd (SBUF-only)
    nc.vector.tensor_tensor(t1[:], A[:, 0:512], cos_t[:, sl],
                            mybir.AluOpType.mult)
    nc.vector.tensor_tensor(t2[:], Bp[:, 0:512], sin_t[:, sl],
                            mybir.AluOpType.mult)
    nc.gpsimd.tensor_tensor(oA, t1[:], t2[:], mybir.AluOpType.subtract)
    nc.vector.tensor_tensor(t3[:], Bp[:, 0:512], cos_t[:, sl],
                            mybir.AluOpType.mult)
    nc.vector.tensor_tensor(t4[:], A[:, 0:512], sin_t[:, sl],
                            mybir.AluOpType.mult)
    nc.gpsimd.tensor_tensor(oB, t3[:], t4[:], mybir.AluOpType.add)
    if sc_i == 0 and e == 1:
        _scatter(nc, rstage, 0, qc, 0, 512)
    if sc_i == 3 and e == 1:
        _scatter(nc, rstage, 0, qc, 1536, 2048)


def _scatter(nc, rstage, base_e, dsts, c0, c1):
    """Scatter the [c0:c1] column span of rstage blocks (base_e, base_e+1)
    into head-contiguous layout: head h x1 -> dsts[h//2][64*(h%2):+32],
    x2 -> +32:+64."""
    for h in range(HPG):
        dt_ = dsts[h // 2]
        po = 64 * (h % 2)
        nc.sync.dma_start(dt_[po:po + 32, c0:c1],
                          rstage[base_e][32 * h:32 * h + 32, c0:c1])
        nc.sync.dma_start(dt_[po + 32:po + 64, c0:c1],
                          rstage[base_e + 1][32 * h:32 * h + 32, c0:c1])


def _attention_zip2(nc, attps, ptp, qlo, qhi, xaccs, kc, qc, v_t,
                    maskT, ident):
    """Both head-pairs of a (small) quarter interleaved tile-by-tile: no
    hp-seam pipeline refill, ACT stays fed.  Used for quarter 0 only (its
    filler/ops pool is free to hold the second pair's accumulators)."""
    t_end = qhi // 128
    pend = {0: None, 1: None}
    for t in range(t_end):
        ktl = slice(t * 128, (t + 1) * 128)
        off = max(qlo, 128 * t)
        n = qhi - off
        base = off % 512
        diag = off == 128 * t
        for hp in (0, 1):
            heads = (2 * hp, 2 * hp + 1)
            sc = attps.tile([128, 1024], F32, tag="sc",
                            name=f"scz_{t}_{hp}")
            for j, h in enumerate(heads):
                hs = slice(64 * j, 64 * j + 64)
                nc.tensor.matmul(
                    sc[:, 512 * j + base:512 * j + base + n], kc[hp][hs, ktl],
                    qc[hp][hs, off:off + n], start=True, stop=not diag)
                if diag:
                    nc.tensor.matmul(
                        sc[:, 512 * j + base:512 * j + base + 128],
                        maskT[:], ident[:], start=False, stop=True)
            pt = ptp.tile([128, 1024], BF16, tag="pt", name=f"ptz_{t}_{hp}")
            scv = sc.rearrange("p (j c) -> p j c", j=2)
            ptv = pt.rearrange("p (j c) -> p j c", j=2)
            nc.scalar.activation(
                ptv[:, :, base:base + n], scv[:, :, base:base + n],
                mybir.ActivationFunctionType.Exp, scale=SCALE / QK_SCALE)
            if pend[hp] is not None:
                _emit_pv(nc, heads, xaccs, v_t, qlo, *pend[hp], t_end)
            pend[hp] = (pt, base, n, off, t)
    for hp in (0, 1):
        _emit_pv(nc, (2 * hp, 2 * hp + 1), xaccs, v_t, qlo, *pend[hp], t_end)


def _prestart_t0(nc, attps, ptp, qh, hp, kc, qc, maskT, ident, stash):
    """Emit the first score tile + exp of pair (qh, hp) early (as a filler
    in the previous pair's endgame) so the ACT pipeline stays warm across
    the seam; _attention_pair picks it up via `stash`."""
    qlo = 512 * qh
    off, n, base = qlo, 512, 0
    diag = qlo == 0
    heads = (2 * hp, 2 * hp + 1)
    sc = attps.tile([128, 1024], F32, tag="sc", name=f"scp_{qh}_{hp}")
    for j, h in enumerate(heads):
        hs = slice(64 * j, 64 * j + 64)
        nc.tensor.matmul(
            sc[:, 512 * j:512 * j + n], kc[hp][hs, 0:128],
            qc[hp][hs, off:off + n], start=True, stop=not diag)
        if diag:
            nc.tensor.matmul(sc[:, 512 * j:512 * j + 128],
                             maskT[:], ident[:], start=False, stop=True)
    pt = ptp.tile([128, 1024], BF16, tag="pt", name=f"ptp_{qh}_{hp}")
    scv = sc.rearrange("p (j c) -> p j c", j=2)
    ptv = pt.rearrange("p (j c) -> p j c", j=2)
    nc.scalar.activation(
        ptv[:, :, base:base + n], scv[:, :, base:base + n],
        mybir.ActivationFunctionType.Exp, scale=SCALE / QK_SCALE)
    stash[(qh, hp)] = (pt, base, n, off, 0)


def _attention_pair(nc, attps, ptp, heads, qlo, qhi, xaccs, kc, qc, v_t,
                    maskT, ident, sched, pend0=None):
    """QK->exp->PV for a pair of heads over q range [qlo, qhi).
    Both heads' scores for one k-tile land in a single 2-bank PSUM tile so
    ONE exp instruction covers them.  Causal masking of diagonal blocks:
    a DVE add of a strict-lower-triangular -1e8 constant onto the diag
    region of the score psum (k > q), so exp underflows those entries to
    exactly 0.  Software-pipelined: PV(t-1) is emitted
    after scores(t)+filler so the PE never waits on exp(t-1) latency."""
    t_end = qhi // 128
    hp = heads[0] // 2
    pend = pend0
    for piece in sched.get(-1, ()):
        piece()
    if pend0 is not None:
        for piece in sched.get(0, ()):
            piece()
    for t in range(1 if pend0 is not None else 0, t_end):
        ktl = slice(t * 128, (t + 1) * 128)
        off = max(qlo, 128 * t)
        n = qhi - off
        base = off % 512
        diag = off == 128 * t
        sc = attps.tile([128, 1024], F32, tag="sc")
        for j, h in enumerate(heads):
            hs = slice(64 * j, 64 * j + 64)
            nc.tensor.matmul(
                sc[:, 512 * j + base:512 * j + base + n], kc[hp][hs, ktl],
                qc[hp][hs, off:off + n], start=True, stop=not diag)
            if diag:
                nc.tensor.matmul(
                    sc[:, 512 * j + base:512 * j + base + 128],
                    maskT[:], ident[:], start=False, stop=True)
        pt = ptp.tile([128, 1024], BF16, tag="pt")
        scv = sc.rearrange("p (j c) -> p j c", j=2)
        ptv = pt.rearrange("p (j c) -> p j c", j=2)
        nc.scalar.activation(
            ptv[:, :, base:base + n], scv[:, :, base:base + n],
            mybir.ActivationFunctionType.Exp, scale=SCALE / QK_SCALE)
        for piece in sched.get(t, ()):
            piece()
        if pend is not None:
            _emit_pv(nc, heads, xaccs, v_t, qlo, *pend, t_end)
        pend = (pt, base, n, off, t)
    _emit_pv(nc, heads, xaccs, v_t, qlo, *pend, t_end)


def _emit_pv(nc, heads, xaccs, v_t, qlo, pt, base, n, off, t, t_end):
    for j, h in enumerate(heads):
        nc.tensor.matmul(
            xaccs[h][:, off - qlo:off - qlo + n],
            v_t[t][:, 65 * h:65 * h + 65],
            pt[:, 512 * j + base:512 * j + base + n],
            start=(t == 0), stop=(t == t_end - 1))


def _body(nc, tc, xT, wqkT, wvT, cos4, sin4, woT, out):
    with tc.tile_pool(name="const", bufs=1) as constp, \
         tc.tile_pool(name="rot", bufs=1) as rotp, \
         tc.tile_pool(name="vsd", bufs=1) as vsd, \
         tc.tile_pool(name="xnorm", bufs=1) as xnp, \
         tc.tile_pool(name="wo", bufs=1) as wop, \
         tc.tile_pool(name="xw", bufs=1) as xw, \
         tc.tile_pool(name="ptp", bufs=5) as ptp, \
         tc.tile_pool(name="nrm", bufs=3) as nrmp, \
         tc.tile_pool(name="oout", bufs=4) as ooutp:
        # maskT/ident implement in-matmul causal masking of diagonal
        # score blocks (adds -1e8 where k > q before the exp)
        maskT = constp.tile([128, 128], BF16)
        make_upper_triangular(nc, maskT[:], val=-1.0e8, diag=False)
        ident = constp.tile([128, 128], BF16)
        make_identity(nc, ident[:])
        ones64 = constp.tile([1, 64], F32)
        nc.vector.memset(ones64[:], 1.0)
        ones64b = constp.tile([1, 64], BF16)
        nc.vector.memset(ones64b[:], 1.0)
        # dummy exp: pulls the ACT table load into phase A (off the first
        # attention quarter's critical path)
        dummy = constp.tile([1, 16], F32)
        nc.scalar.activation(dummy[:], ones64[:, 0:16],
                             mybir.ActivationFunctionType.Exp, scale=1.0)
        # head-contiguous rotated Q/K: qc[i] holds heads 2i, 2i+1 with each
        # head's 64 features (x1;x2) contiguous on partitions
        qc = [rotp.tile([128, S], BF16, name=f"qc_{i}") for i in range(2)]
        kc = [rotp.tile([128, S], BF16, name=f"kc_{i}") for i in range(2)]
        # persistent RoPE output staging (block layout: QA QB KA KB)
        rstage = [rotp.tile([128, S], BF16, name=f"rs_{e}") for e in range(4)]
        v_t = [vsd.tile([128, 260], BF16, name=f"v_{st}") for st in range(ST)]
        xn = [xnp.tile([128, S], BF16, name=f"xn_{hp}") for hp in range(2)]
        wo_t = [wop.tile([128, D], BF16, name=f"wo_{d2}") for d2 in range(2)]

        # d-chunked fp8 hi/lo operands live in single big tiles (few, large
        # DMAs: the hwdge descriptor unit costs ~0.6us per DMA instruction)
        x_t = xw.tile([128, DT, 2, S], F8, name="xbig")
        wqk_t = xw.tile([128, DT, 2, 512], F8, name="wqkbig")
        wv_t = xw.tile([128, DT, 2, 256], F8, name="wvbig")
        cos_t = xw.tile([128, S], F32)
        sin_t = xw.tile([128, S], F32)
        xTr = xT.rearrange("(d p) j c -> p d j c", d=DT)
        wqkr = wqkT.rearrange("(d p) j c -> p d j c", d=DT)
        wvr = wvT.rearrange("(d p) j c -> p d j c", d=DT)
        # loads in consumption order: wqk halves zipped with x first-halves
        nc.sync.dma_start(wqk_t[:, 0:4], wqkr[:, 0:4])
        nc.sync.dma_start(x_t[:, 0:2, 0, 0:1024], xTr[:, 0:2, 0, 0:1024])
        nc.sync.dma_start(x_t[:, 0:2, 1, 0:1024], xTr[:, 0:2, 1, 0:1024])
        nc.sync.dma_start(wqk_t[:, 4:8], wqkr[:, 4:8])
        for dp in range(1, 4):
            nc.sync.dma_start(x_t[:, 2 * dp:2 * dp + 2, 0, 0:1024],
                              xTr[:, 2 * dp:2 * dp + 2, 0, 0:1024])
            nc.sync.dma_start(x_t[:, 2 * dp:2 * dp + 2, 1, 0:1024],
                              xTr[:, 2 * dp:2 * dp + 2, 1, 0:1024])
        nc.sync.dma_start(cos_t[:], cos4[:])
        nc.sync.dma_start(sin_t[:], sin4[:])
        nc.sync.dma_start(wv_t[:], wvr[:])
        for dp in range(4):
            for j in range(2):
                nc.sync.dma_start(x_t[:, 2 * dp:2 * dp + 2, j, 1024:2048],
                                  xTr[:, 2 * dp:2 * dp + 2, j, 1024:2048])
        for d2 in range(2):
            nc.sync.dma_start(wo_t[d2][:], woT[d2 * 128:(d2 + 1) * 128, :])

        # ============ Phase A: projections (all 4 chunks) ============
        with tc.tile_pool(name="ropet", bufs=4) as ropet:
            with tc.tile_pool(name="qkps", bufs=3, space="PSUM") as qkps, \
                 tc.tile_pool(name="vps", bufs=2, space="PSUM") as vps:
                for sc_i in range(4):
                    pools = (
                        lambda e, s=sc_i: qkps.tile(
                            [128, 512], F32, tag=f"qk{e % 2}",
                            name=f"qk{e}_{s}"),
                        lambda st: vps.tile([128, 256], F32, tag="vps",
                                            name=f"pv_{st}"),
                    )
                    # V of chunk 3 is deferred into the A3 attention loop
                    # (its PVs only need v_t[12..15] near the t-loop end).
                    for piece in _proj_chunk_pieces(
                            nc, sc_i, pools, x_t, wqk_t, wv_t, cos_t, sin_t,
                            qc, rstage, v_t, ropet, include_v=(sc_i < 3)):
                        piece()
                    if sc_i == 1:
                        # A3's first 8 k-tiles only need kc columns 0:1024
                        _scatter(nc, rstage, 2, kc, 0, 1024)
                    if sc_i == 3:
                        _scatter(nc, rstage, 2, kc, 1024, 2048)
                        _scatter(nc, rstage, 0, qc, 512, 1536)

            # ======== Phase B: attention quarters (3,2,1,0) + out-proj ====
            # Descending order puts the smallest quarter last (short tail);
            # the out-projection of each processed quarter becomes PE filler
            # work inside the next quarter's attention loop.
            # psum: sc [128,1024]x2 + xacc [65,512]x3 + ops [128,512]x1 = 8.
            # xacc is 3-deep so a new head-pair's accumulators never wait on
            # the previous pair's norm reads (the boundary serializer).
            with tc.tile_pool(name="attps", bufs=2, space="PSUM") as attps, \
                 tc.tile_pool(name="xaccps", bufs=2, space="PSUM") as xaccps, \
                 tc.tile_pool(name="ops", bufs=2, space="PSUM") as opsp:
                ops_alloc = (lambda st: opsp.tile([128, 512], F32, tag="ops",
                                                  name=f"dpv_{st}"))
                seq = [(0, 0), (0, 1), (3, 0), (3, 1), (2, 0), (2, 1),
                       (1, 0), (1, 1)]
                prestash = {}
                prev_qh = None
                for qh in (0, 3, 2, 1):
                    qlo, qhi = 512 * qh, 512 * (qh + 1)
                    t_end = 4 * (qh + 1)
                    # Per-hp filler schedules (tile -> pieces): deferred
                    # V-projections are EAGER (one per tile from t=0, their
                    # PVs consume them later in the same loop); out-proj
                    # units of the previous quarter are spread evenly.
                    sched = [{}, {}]
                    if qh == 3:
                        # all 8 pieces in hp0, positioned as late as each
                        # PV dependency allows (the diag region is where ACT
                        # outpaces PE and needs PE filler)
                        vp = list(_v_pieces(nc, range(12, 16), ops_alloc,
                                            x_t, wv_t, v_t))
                        vpos = [0, 8, 9, 10, 11, 12, 13, 14]
                        for i, p in enumerate(vp):
                            sched[0].setdefault(vpos[i], []).append(p)
                    if prev_qh is not None:
                        ou = list(_outproj_units(nc, opsp, ooutp, xn, wo_t,
                                                 out, prev_qh,
                                                 act_c1=False))
                        # one unit at the hp-boundary warmup, the rest in the
                        # trailing diag region where ACT outpaces PE
                        pos = [t_end - 4, t_end - 3, t_end - 2, t_end - 1]
                        for k in range(2):
                            for i, p in enumerate(ou[4 * k:4 * k + 4]):
                                sched[k].setdefault(pos[i], []).append(p)

                    for hp in range(2):
                        # last filler slot: prestart the NEXT pair's first
                        # score tile + exp so ACT never idles across seams
                        i = seq.index((qh, hp))
                        if i + 1 < len(seq):
                            nqh, nhp = seq[i + 1]
                            sched[hp].setdefault(t_end - 1, []).append(
                                lambda nqh=nqh, nhp=nhp: _prestart_t0(
                                    nc, attps, ptp, nqh, nhp, kc, qc,
                                    maskT, ident, prestash))
                        heads = (2 * hp, 2 * hp + 1)
                        xaccs = {}
                        for h in heads:
                            xaccs[h] = xaccps.tile([65, 512], F32, tag="xacc",
                                                   name=f"xacc_{qh}_{h}")
                        _attention_pair(nc, attps, ptp, heads, qlo, qhi,
                                        xaccs, kc, qc, v_t, maskT, ident,
                                        sched[hp],
                                        pend0=prestash.pop((qh, hp), None))
                        if qh == 1 and hp == 1:
                            # pre-start the tail's d2=0 accumulations (they
                            # only need xn[0]) so the PE overlaps the final
                            # norm ladder instead of waiting behind it
                            tail_p1, tail_p2 = _tail_outproj(
                                nc, attps, opsp, ooutp, xn, wo_t, out)
                            tail_p1()
                        for h in heads:
                            xacc = xaccs[h]
                            # stash the accumulators to SBUF on ACT first:
                            # this releases the xacc PSUM bank in ~0.6us so
                            # the next pair's PVs never wait on the norm
                            # ladder (which otherwise holds the bank ~5us -
                            # reciprocal on a [1,512] single-partition AP is
                            # ~3.3us on DVE).  The ladder then runs from
                            # SBUF fully off the PE critical path.
                            xs = nrmp.tile([65, 512], F32, tag="xs")
                            # copy on DVE: ACT (exp) is the busier engine
                            # in the attention steady state
                            nc.vector.tensor_scalar_mul(xs[:], xacc[:], 1.0)
                            xsrc = xs
                            # den must sit at partition base 0: the
                            # custom-DVE reciprocal_approx_fast corrupts
                            # data when its input AP has a non-zero base
                            # partition (verified on hw), so stage row 64
                            # down with a cheap DVE copy first.
                            den = nrmp.tile([1, 512], F32, tag="den")
                            nc.vector.tensor_scalar_mul(den[:], xsrc[64:65, :],
                                                        1.0)
                            recip = nrmp.tile([1, 512], F32, tag="recip")
                            nc.vector.reciprocal_approx_fast(recip[:], den[:])
                            rb = nrmp.tile([64, 512], F32, tag="rb")
                            nc.gpsimd.partition_broadcast(rb[:], recip[:])
                            dst = xn[h // 2][64 * (h % 2):64 * (h % 2) + 64, :]
                            nc.vector.tensor_tensor(
                                dst[:, qlo:qhi], xsrc[0:64, :], rb[:],
                                mybir.AluOpType.mult)
                    prev_qh = qh
                # tail: rest of the last quarter's out-projection
                tail_p2()


def _to_bf16(a):
    import ml_dtypes
    return np.ascontiguousarray(a.astype(ml_dtypes.bfloat16))


def _to_hilo8(a, scale):
    """[R, C] f32 -> [R, 2, C] fp8 e4m3 (hi, residual-lo) after scaling."""
    import ml_dtypes
    E4 = ml_dtypes.float8_e4m3
    a = np.asarray(a, np.float32) * scale
    hi = a.astype(E4)
    lo = (a - hi.astype(np.float32)).astype(E4)
    return np.ascontiguousarray(np.stack([hi, lo], axis=1))


def prepare_in_maps(inputs, positions, W_in, W_out):
    """Build per-core input shards (all host-side numpy prep)."""
    inputs = np.ascontiguousarray(inputs, dtype=np.float32)
    W_in = np.ascontiguousarray(W_in, dtype=np.float32)
    W_out = np.ascontiguousarray(W_out, dtype=np.float32)
    positions = np.asarray(positions)

    inv_freq = 1.0 / (ROPE_BASE ** (np.arange(0, HD, 2, dtype=np.float32) / HD))
    in_maps = []
    for core in range(N_CORES):
        b, g = divmod(core, G)
        heads = [g * HPG + h for h in range(HPG)]

        xTb = inputs[b].T                                          # [D, S]

        # RoPE tables [128, S]: rows = 4 copies of the 32 freqs
        ang = positions[b].astype(np.float32)[None, :] * inv_freq[:, None]
        cos4 = np.tile(np.cos(ang), (4, 1)).astype(np.float32)
        sin4 = np.tile(np.sin(ang), (4, 1)).astype(np.float32)

        # QK weight blocks: QA/QB/KA/KB, each 128 rows (4 heads x 32)
        def rows(base_off):
            idx = []
            for h in heads:
                idx.extend(h * 3 * HD + base_off + f for f in range(32))
            return idx
        qk_idx = rows(0) + rows(32) + rows(64) + rows(96)
        wqkT = W_in[qk_idx].T                                      # [D, 512]

        v_idx = []
        for h in heads:
            v_idx.extend(h * 3 * HD + 2 * HD + f for f in range(HD))
        wvT = W_in[v_idx].T                                        # [D, 256]

        # W_out columns for this group's feature slice, transposed
        dsl = [h * HD + f for h in heads for f in range(HD)]
        woT = W_out[:, dsl].T                                      # [256, D]

        in_maps.append({
            "xT": _to_hilo8(xTb, X_SCALE),
            "wqkT": _to_hilo8(wqkT, W_SCALE),
            "wvT": _to_hilo8(wvT, W_SCALE),
            "cos4": cos4, "sin4": sin4, "woT": _to_bf16(woT),
        })
    return in_maps


def assemble_output(results):
    """Sum the 4 per-group partials (bf16) for each batch."""
    out = np.zeros((B, S, D), dtype=np.float32)
    for core in range(N_CORES):
        b = core // G
        out[b] += np.asarray(results[core]["out"], dtype=np.float32)
    return out


_NC_CACHE = {}


def get_nc():
    if "nc" not in _NC_CACHE:
        _NC_CACHE["nc"] = build_nc()
    return _NC_CACHE["nc"]


def kernel(inputs, positions, W_in, W_out):
    nc = get_nc()
    in_maps = prepare_in_maps(inputs, positions, W_in, W_out)
    res = bass_utils.run_bass_kernel_spmd(
        nc, in_maps, core_ids=list(range(N_CORES)))
    return assemble_output(res.results)



# revision 20
# speedup vs baseline: 1.0220x; 1.0026x over previous
"""Trainium2 Bass kernel for nn_AttentionBlock (B=2, S=2048, D=1024, H=16, HD=64).

Sharding: 8 cores = 2 batches x 4 head-groups (4 heads each).
Each core computes, for its (batch b, head-group g):
  - fused QK projection (RoPE'd, feature-transposed layout) + V projection
  - causal attention for its 4 heads (scores computed transposed, softmax
    denominator via an appended ones-column in the PV matmul)
  - a PARTIAL output projection: x_g @ W_out[:, d-slice].T  -> [S, D] partial
The host sums the 4 partials per batch (linear unshard step) - no on-device
collectives needed.

Schedule: phase A runs the QKV projections for all four 512-column chunks
(V of chunk 3 deferred), then attention quarters are processed in order
(0, 3, 2, 1).  Within a quarter the t-loop is software-pipelined
(scores(t) -> exp(t) -> PV(t-1)); both heads of a pair share one 2-bank
PSUM score tile so a single exp instruction covers them; causal masking is
done inside the scores matmul group (a -1e8 upper-triangular add) so no
vector-engine op sits on the critical chain.  After each pair, the xacc
accumulators are stashed PSUM->SBUF with one DVE copy (releases the banks
in ~0.6us so the next pair's PVs never wait), and the softmax denominators
are inverted off-path with the single-pass custom-DVE approx reciprocal
(input staged to partition base 0 - the op corrupts data for non-zero base
partitions).  The out-projection of each
finished quarter and the deferred V tiles are emitted as PE "filler"
pieces inside the next quarter's loop, positioned in the trailing diagonal
region where the ACT engine otherwise outpaces the PE; each pair's final
filler slot also pre-starts the NEXT pair's first score tile + exp so the
ACT pipeline stays warm across seams.
PSUM: projections 6+2 banks; attention 4 (scores) + 2 (xacc) + 2 (outproj).

Dtypes: the QKV projections run as fp8(e4m3) DoubleRow matmuls on hi/lo
split operands (x*8, W_in*32: Wh(xh+xl) + Wl xh), which both doubles PE
throughput and carries ~11 effective mantissa bits; everything downstream
(RoPE output, scores, pt, v, xn, W_out) is bf16 with fp32 PSUM
accumulation; output partials are written bf16 (rescaled by 1/256).

Self-contained: hardcodes all shapes; imports only concourse + numpy.
"""
import math

import numpy as np

import concourse.bass as bass  # noqa: F401
import concourse.bacc as bacc
import concourse.mybir as mybir
import concourse.tile as tile
from concourse import bass_utils
from concourse.masks import make_identity, make_upper_triangular

B, S, D, H = 2, 2048, 1024, 16
HD = D // H            # 64
G = 4                  # head-groups (cores per batch)
HPG = H // G           # 4 heads per group
N_CORES = 8
ROPE_BASE = 10000.0
F32 = mybir.dt.float32
F32R = mybir.dt.float32r
BF16 = mybir.dt.bfloat16
F8 = mybir.dt.float8e4
DR = mybir.MatmulPerfMode.DoubleRow

KT = S // 128          # 16 k-tiles of 128
ST = S // 128          # 16 s-tiles
DT = D // 128          # 8 d-chunks
X_SCALE = 8.0          # fp8 pre-scale on x
W_SCALE = 32.0         # fp8 pre-scale on W_in (lifts W out of e4m3 subnormals)
QK_SCALE = (X_SCALE * W_SCALE) ** 2   # scores carry (x*W)^2 scaling
V_SCALE = X_SCALE * W_SCALE           # v / xn / out-proj psum carry this
SCALE = 1.0 / math.sqrt(HD)


def build_nc():
    nc = bacc.Bacc("TRN2", target_bir_lowering=False, debug=False,
                   num_devices=N_CORES)

    # x (x8) and projection weights (x32) in fp8 e4m3 hi/lo pairs: the
    # projections run as DoubleRow fp8 matmuls (Wh(xh+xl) + Wl xh), which
    # stream two contraction rows per cycle on the PE.
    xT = nc.dram_tensor("xT", [D, 2, S], F8, kind="ExternalInput").ap()
    # QK weights, transposed+permuted: columns = [QA|QB|KA|KB] of 128 each.
    wqkT = nc.dram_tensor("wqkT", [D, 2, 512], F8, kind="ExternalInput").ap()
    # V weights, transposed: columns = 4 heads x 64 feats.
    wvT = nc.dram_tensor("wvT", [D, 2, 256], F8, kind="ExternalInput").ap()
    # RoPE tables, [128, S]: rows = 4x (32 freqs).
    cos4 = nc.dram_tensor("cos4", [128, S], F32, kind="ExternalInput").ap()
    sin4 = nc.dram_tensor("sin4", [128, S], F32, kind="ExternalInput").ap()
    # W_out columns for this group's features, transposed: [256, D].
    woT = nc.dram_tensor("woT", [256, D], BF16, kind="ExternalInput").ap()
    out = nc.dram_tensor("out", [S, D], BF16, kind="ExternalOutput").ap()

    with tile.TileContext(nc) as tc:
        _body(nc, tc, xT, wqkT, wvT, cos4, sin4, woT, out)
    nc.compile()
    return nc


def _outproj_units(nc, opsp, ooutp, xn, wo_t, out, qh, tail=False,
                   act_c1=False):
    """Yield closures, each emitting one (st, c) out-projection piece
    (2 matmuls + psum->sbuf copy + dma on the second half).  Used as PE
    filler work.  The tail variant allocates 2-bank psum tiles (from the
    then-idle attention score pool) so both halves run concurrently, and
    puts half the copies on ACT so the final drain parallelizes."""
    state = {}
    pstate = {}

    def unit(st, c):
        stl = slice(st * 128, (st + 1) * 128)
        if st not in state:
            state[st] = ooutp.tile([128, 1024], BF16, tag="ot",
                                   name=f"ot_{qh}_{st}")
        ot = state[st]
        if tail:
            if st not in pstate:
                pstate[st] = opsp.tile([128, 1024], F32, tag="sc",
                                       name=f"pot_{qh}_{st}")
            po = pstate[st][:, c * 512:(c + 1) * 512]
        else:
            po = opsp.tile([128, 512], F32, tag="ops",
                           name=f"po_{qh}_{st}_{c}")
        for d2 in range(2):
            nc.tensor.matmul(
                po, xn[d2][:, stl], wo_t[d2][:, c * 512:(c + 1) * 512],
                start=(d2 == 0), stop=(d2 == 1))
        if c == 1 and (tail or act_c1):
            # tail copies on ACT so the final drain parallelizes with DVE
            nc.scalar.mul(ot[:, c * 512:(c + 1) * 512], po, 1.0 / V_SCALE)
        else:
            nc.vector.tensor_scalar_mul(ot[:, c * 512:(c + 1) * 512], po,
                                        1.0 / V_SCALE)
        nc.sync.dma_start(out[stl, c * 512:(c + 1) * 512],
                          ot[:, c * 512:(c + 1) * 512])

    cs = (1, 0) if tail else (0, 1)
    for st in range(4 * qh, 4 * qh + 4):
        for c in cs:
            yield (lambda st=st, c=c: unit(st, c))


def _tail_outproj(nc, attps, opsp, ooutp, xn, wo_t, out):
    """Two-part tail for quarter 1 (st 4..7): part1 emits the d2=0 halves of
    st4/st5/st6 (xn[0]-only dependency, runnable during the final norm
    ladder); part2 closes them and runs st7, copies split ACT(c1)/DVE(c0).
    st6 borrows the (tail-idle) ops pool so it doesn't wait on st4/st5's
    sc-slot copies; st7 reuses st4's sc slot.  Each finished half DMAs
    immediately (transfer latency beats issue-slot count at the drain)."""
    state = {}
    pstate = {}

    def ensure(st):
        if st not in state:
            state[st] = ooutp.tile([128, 1024], BF16, tag="ot",
                                   name=f"ot_t_{st}")
            if st == 6:
                pstate[st] = [opsp.tile([128, 512], F32, tag="ops",
                                        name=f"pot_t_{st}_{c}")
                              for c in range(2)]
            else:
                pt_ = attps.tile([128, 1024], F32, tag="sc",
                                 name=f"pot_t_{st}")
                pstate[st] = [pt_[:, 0:512], pt_[:, 512:1024]]

    def mm(st, c, d2, start, stop):
        stl = slice(st * 128, (st + 1) * 128)
        po = pstate[st][c]
        nc.tensor.matmul(po, xn[d2][:, stl],
                         wo_t[d2][:, c * 512:(c + 1) * 512],
                         start=start, stop=stop)

    def finish(st, c):
        stl = slice(st * 128, (st + 1) * 128)
        po = pstate[st][c]
        ot = state[st]
        if c == 1:
            nc.scalar.mul(ot[:, c * 512:(c + 1) * 512], po, 1.0 / V_SCALE)
        else:
            nc.vector.tensor_scalar_mul(ot[:, c * 512:(c + 1) * 512], po,
                                        1.0 / V_SCALE)
        nc.sync.dma_start(out[stl, c * 512:(c + 1) * 512],
                          ot[:, c * 512:(c + 1) * 512])

    def part1():
        for st in (4, 5, 6):
            ensure(st)
            for c in (0, 1):
                mm(st, c, 0, True, False)

    def part2():
        for st in (4, 5):
            for c in (1, 0):
                mm(st, c, 1, False, True)
                finish(st, c)
        for c in (1, 0):
            mm(6, c, 1, False, True)
            finish(6, c)
        ensure(7)
        for c in (1, 0):
            mm(7, c, 0, True, False)
            mm(7, c, 1, False, True)
            finish(7, c)

    return part1, part2


def _v_pieces(nc, sts, v_alloc, x_t, wv_t, v_t):
    """Yield closures emitting the V-projection of s-tiles `sts`, two pieces
    (6 fp8 DoubleRow matmuls) each."""
    ps = {}
    TERMS = ((0, 0), (0, 1), (1, 0))  # (W hi/lo, x hi/lo)

    def v_piece(st, half):
        if half == 0:
            ps[st] = v_alloc(st)
        pv = ps[st]
        stl = slice(st * 128, (st + 1) * 128)
        for i in range(6 * half, 6 * half + 6):
            (wh, xh), dp = TERMS[i // 4], i % 4
            nc.tensor.matmul(
                pv[:, 0:256],
                x_t[:, 2 * dp:2 * dp + 2, xh, stl],
                wv_t[:, 2 * dp:2 * dp + 2, wh, :],
                start=(i == 0), stop=(i == 11), perf_mode=DR)
        if half == 1:
            nc.scalar.copy(
                v_t[st][:].rearrange("p (h f) -> p h f", h=4)[:, :, 0:64],
                pv[:, 0:256].rearrange("p (h f) -> p h f", h=4))
            nc.gpsimd.memset(
                v_t[st][:].rearrange("p (h f) -> p h f", h=4)[:, :, 64:65],
                1.0)

    for st in sts:
        for half in range(2):
            yield (lambda st=st, half=half: v_piece(st, half))


def _proj_chunk_pieces(nc, sc_i, pools, x_t, wqk_t, wv_t, cos_t, sin_t,
                       qc, rstage, v_t, ropet, include_v=True):
    """Yield closures emitting projection chunk sc_i piecewise (~2 matmuls
    per piece) so it can be used as PE filler inside attention loops.
    pools: (qk_alloc, v_alloc) -> psum AP factories."""
    qk_alloc, v_alloc = pools
    sl = slice(sc_i * 512, (sc_i + 1) * 512)
    ps = {}

    # 3 hi/lo terms x 4 d-pairs of fp8 DoubleRow matmuls, 2 per piece.
    # terms: (hi W x hi x), (hi W x lo x), (lo W x hi x)
    TERMS = ((0, 0), (0, 1), (1, 0))

    def qk_piece(e, pc):
        if pc == 0:
            ps[e] = qk_alloc(e)
        p = ps[e]
        for i in (2 * pc, 2 * pc + 1):
            (wh, xh), dp = TERMS[i // 4], i % 4
            nc.tensor.matmul(
                p[:, 0:512],
                wqk_t[:, 2 * dp:2 * dp + 2, wh, e * 128:(e + 1) * 128],
                x_t[:, 2 * dp:2 * dp + 2, xh, sl],
                start=(i == 0), stop=(i == 11), perf_mode=DR)
        if pc == 5 and e % 2 == 1:
            _emit_rope(nc, sc_i, sl, e, ps, rstage, cos_t, sin_t, qc, ropet)

    # chunk 3 runs K blocks first: the last rope then covers only Q3,
    # whose consumer (quarter A3) runs second - the first quarter's psum
    # WAR on these banks clears sooner
    e_order = (2, 3, 0, 1) if sc_i == 3 else (0, 1, 2, 3)
    for e in e_order:
        for pc in range(6):
            yield (lambda e=e, pc=pc: qk_piece(e, pc))
    if include_v:
        yield from _v_pieces(nc, range(4 * sc_i, 4 * (sc_i + 1)), v_alloc,
                             x_t, wv_t, v_t)


def _emit_rope(nc, sc_i, sl, e, ps, rstage, cos_t, sin_t, qc, ropet):
    """RoPE for the (e-1, e) block pair, writing into the persistent
    rstage[e] tiles (block layout, full S).  Scatter DMAs into qc/kc are
    batched separately (see _scatter) - except Q of chunk 3, which quarter
    A3 needs immediately after phase A."""
    A, Bp = ps[e - 1], ps[e]
    oA = rstage[e - 1][:, sl]
    oB = rstage[e][:, sl]
    t1 = ropet.tile([128, 512], F32, tag="t1")
    t2 = ropet.tile([128, 512], F32, tag="t2")
    t3 = ropet.tile([128, 512], F32, tag="t3")
    t4 = ropet.tile([128, 512], F32, tag="t4")
    # oA = A*cos - B*sin ; oB = B*cos + A*sin
    # products on DVE (PSUM reads); combines on GpSimd (SBUF-only)
    nc.vector.tensor_tensor(t1[:], A[:, 0:512], cos_t[:, sl],
                            mybir.AluOpType.mult)
    nc.vector.tensor_tensor(t2[:], Bp[:, 0:512], sin_t[:, sl],
                            mybir.AluOpType.mult)
    nc.gpsimd.tensor_tensor(oA, t1[:], t2[:], mybir.AluOpType.subtract)
    nc.vector.tensor_tensor(t3[:], Bp[:, 0:512], cos_t[:, sl],
                            mybir.AluOpType.mult)
    nc.vector.tensor_tensor(t4[:], A[:, 0:512], sin_t[:, sl],
                            mybir.AluOpType.mult)
    nc.gpsimd.tensor_tensor(oB, t3[:], t4[:], mybir.AluOpType.add)
    if sc_i == 0 and e == 1:
        _scatter(nc, rstage, 0, qc, 0, 512)
    if sc_i == 3 and e == 1:
        _scatter(nc, rstage, 0, qc, 1536, 2048)


def _scatter(nc, rstage, base_e, dsts, c0, c1):
    """Scatter the [c0:c1] column span of rstage blocks (base_e, base_e+1)
    into head-contiguous layout: head h x1 -> dsts[h//2][64*(h%2):+32],
    x2 -> +32:+64."""
    for h in range(HPG):
        dt_ = dsts[h // 2]
        po = 64 * (h % 2)
        nc.sync.dma_start(dt_[po:po + 32, c0:c1],
                          rstage[base_e][32 * h:32 * h + 32, c0:c1])
        nc.sync.dma_start(dt_[po + 32:po + 64, c0:c1],
                          rstage[base_e + 1][32 * h:32 * h + 32, c0:c1])


def _attention_zip2(nc, attps, ptp, qlo, qhi, xaccs, kc, qc, v_t,
                    maskT, ident):
    """Both head-pairs of a (small) quarter interleaved tile-by-tile: no
    hp-seam pipeline refill, ACT stays fed.  Used for quarter 0 only (its
    filler/ops pool is free to hold the second pair's accumulators)."""
    t_end = qhi // 128
    pend = {0: None, 1: None}
    for t in range(t_end):
        ktl = slice(t * 128, (t + 1) * 128)
        off = max(qlo, 128 * t)
        n = qhi - off
        base = off % 512
        diag = off == 128 * t
        for hp in (0, 1):
            heads = (2 * hp, 2 * hp + 1)
            sc = attps.tile([128, 1024], F32, tag="sc",
                            name=f"scz_{t}_{hp}")
            for j, h in enumerate(heads):
                hs = slice(64 * j, 64 * j + 64)
                nc.tensor.matmul(
                    sc[:, 512 * j + base:512 * j + base + n], kc[hp][hs, ktl],
                    qc[hp][hs, off:off + n], start=True, stop=not diag)
                if diag:
                    nc.tensor.matmul(
                        sc[:, 512 * j + base:512 * j + base + 128],
                        maskT[:], ident[:], start=False, stop=True)
            pt = ptp.tile([128, 1024], BF16, tag="pt", name=f"ptz_{t}_{hp}")
            scv = sc.rearrange("p (j c) -> p j c", j=2)
            ptv = pt.rearrange("p (j c) -> p j c", j=2)
            nc.scalar.activation(
                ptv[:, :, base:base + n], scv[:, :, base:base + n],
                mybir.ActivationFunctionType.Exp, scale=SCALE / QK_SCALE)
            if pend[hp] is not None:
                _emit_pv(nc, heads, xaccs, v_t, qlo, *pend[hp], t_end)
            pend[hp] = (pt, base, n, off, t)
    for hp in (0, 1):
        _emit_pv(nc, (2 * hp, 2 * hp + 1), xaccs, v_t, qlo, *pend[hp], t_end)


def _prestart_t0(nc, attps, ptp, qh, hp, kc, qc, maskT, ident, stash):
    """Emit the first score tile + exp of pair (qh, hp) early (as a filler
    in the previous pair's endgame) so the ACT pipeline stays warm across
    the seam; _attention_pair picks it up via `stash`."""
    qlo = 512 * qh
    off, n, base = qlo, 512, 0
    diag = qlo == 0
    heads = (2 * hp, 2 * hp + 1)
    sc = attps.tile([128, 1024], F32, tag="sc", name=f"scp_{qh}_{hp}")
    for j, h in enumerate(heads):
        hs = slice(64 * j, 64 * j + 64)
        nc.tensor.matmul(
            sc[:, 512 * j:512 * j + n], kc[hp][hs, 0:128],
            qc[hp][hs, off:off + n], start=True, stop=not diag)
        if diag:
            nc.tensor.matmul(sc[:, 512 * j:512 * j + 128],
                             maskT[:], ident[:], start=False, stop=True)
    pt = ptp.tile([128, 1024], BF16, tag="pt", name=f"ptp_{qh}_{hp}")
    scv = sc.rearrange("p (j c) -> p j c", j=2)
    ptv = pt.rearrange("p (j c) -> p j c", j=2)
    nc.scalar.activation(
        ptv[:, :, base:base + n], scv[:, :, base:base + n],
        mybir.ActivationFunctionType.Exp, scale=SCALE / QK_SCALE)
    stash[(qh, hp)] = (pt, base, n, off, 0)


def _attention_pair(nc, attps, ptp, heads, qlo, qhi, xaccs, kc, qc, v_t,
                    maskT, ident, sched, pend0=None):
    """QK->exp->PV for a pair of heads over q range [qlo, qhi).
    Both heads' scores for one k-tile land in a single 2-bank PSUM tile so
    ONE exp instruction covers them.  Causal masking of diagonal blocks:
    a DVE add of a strict-lower-triangular -1e8 constant onto the diag
    region of the score psum (k > q), so exp underflows those entries to
    exactly 0.  Software-pipelined: PV(t-1) is emitted
    after scores(t)+filler so the PE never waits on exp(t-1) latency."""
    t_end = qhi // 128
    hp = heads[0] // 2
    pend = pend0
    for piece in sched.get(-1, ()):
        piece()
    if pend0 is not None:
        for piece in sched.get(0, ()):
            piece()
    for t in range(1 if pend0 is not None else 0, t_end):
        ktl = slice(t * 128, (t + 1) * 128)
        off = max(qlo, 128 * t)
        n = qhi - off
        base = off % 512
        diag = off == 128 * t
        sc = attps.tile([128, 1024], F32, tag="sc")
        for j, h in enumerate(heads):
            hs = slice(64 * j, 64 * j + 64)
            nc.tensor.matmul(
                sc[:, 512 * j + base:512 * j + base + n], kc[hp][hs, ktl],
                qc[hp][hs, off:off + n], start=True, stop=not diag)
            if diag:
                nc.tensor.matmul(
                    sc[:, 512 * j + base:512 * j + base + 128],
                    maskT[:], ident[:], start=False, stop=True)
        pt = ptp.tile([128, 1024], BF16, tag="pt")
        scv = sc.rearrange("p (j c) -> p j c", j=2)
        ptv = pt.rearrange("p (j c) -> p j c", j=2)
        nc.scalar.activation(
            ptv[:, :, base:base + n], scv[:, :, base:base + n],
            mybir.ActivationFunctionType.Exp, scale=SCALE / QK_SCALE)
        for piece in sched.get(t, ()):
            piece()
        if pend is not None:
            _emit_pv(nc, heads, xaccs, v_t, qlo, *pend, t_end)
        pend = (pt, base, n, off, t)
    _emit_pv(nc, heads, xaccs, v_t, qlo, *pend, t_end)


def _emit_pv(nc, heads, xaccs, v_t, qlo, pt, base, n, off, t, t_end):
    for j, h in enumerate(heads):
        nc.tensor.matmul(
            xaccs[h][:, off - qlo:off - qlo + n],
            v_t[t][:, 65 * h:65 * h + 65],
            pt[:, 512 * j + base:512 * j + base + n],
            start=(t == 0), stop=(t == t_end - 1))


def _body(nc, tc, xT, wqkT, wvT, cos4, sin4, woT, out):
    with tc.tile_pool(name="const", bufs=1) as constp, \
         tc.tile_pool(name="rot", bufs=1) as rotp, \
         tc.tile_pool(name="vsd", bufs=1) as vsd, \
         tc.tile_pool(name="xnorm", bufs=1) as xnp, \
         tc.tile_pool(name="wo", bufs=1) as wop, \
         tc.tile_pool(name="xw", bufs=1) as xw, \
         tc.tile_pool(name="ptp", bufs=5) as ptp, \
         tc.tile_pool(name="nrm", bufs=3) as nrmp, \
         tc.tile_pool(name="oout", bufs=4) as ooutp:
        # maskT/ident implement in-matmul causal masking of diagonal
        # score blocks (adds -1e8 where k > q before the exp)
        maskT = constp.tile([128, 128], BF16)
        make_upper_triangular(nc, maskT[:], val=-1.0e8, diag=False)
        ident = constp.tile([128, 128], BF16)
        make_identity(nc, ident[:])
        ones64 = constp.tile([1, 64], F32)
        nc.vector.memset(ones64[:], 1.0)
        ones64b = constp.tile([1, 64], BF16)
        nc.vector.memset(ones64b[:], 1.0)
        # dummy exp: pulls the ACT table load into phase A (off the first
        # attention quarter's critical path)
        dummy = constp.tile([1, 16], F32)
        nc.scalar.activation(dummy[:], ones64[:, 0:16],
                             mybir.ActivationFunctionType.Exp, scale=1.0)
        # head-contiguous rotated Q/K: qc[i] holds heads 2i, 2i+1 with each
        # head's 64 features (x1;x2) contiguous on partitions
        qc = [rotp.tile([128, S], BF16, name=f"qc_{i}") for i in range(2)]
        kc = [rotp.tile([128, S], BF16, name=f"kc_{i}") for i in range(2)]
        # persistent RoPE output staging (block layout: QA QB KA KB)
        rstage = [rotp.tile([128, S], BF16, name=f"rs_{e}") for e in range(4)]
        v_t = [vsd.tile([128, 260], BF16, name=f"v_{st}") for st in range(ST)]
        xn = [xnp.tile([128, S], BF16, name=f"xn_{hp}") for hp in range(2)]
        wo_t = [wop.tile([128, D], BF16, name=f"wo_{d2}") for d2 in range(2)]

        # d-chunked fp8 hi/lo operands live in single big tiles (few, large
        # DMAs: the hwdge descriptor unit costs ~0.6us per DMA instruction)
        x_t = xw.tile([128, DT, 2, S], F8, name="xbig")
        wqk_t = xw.tile([128, DT, 2, 512], F8, name="wqkbig")
        wv_t = xw.tile([128, DT, 2, 256], F8, name="wvbig")
        cos_t = xw.tile([128, S], F32)
        sin_t = xw.tile([128, S], F32)
        xTr = xT.rearrange("(d p) j c -> p d j c", d=DT)
        wqkr = wqkT.rearrange("(d p) j c -> p d j c", d=DT)
        wvr = wvT.rearrange("(d p) j c -> p d j c", d=DT)
        # loads in consumption order: wqk halves zipped with x first-halves
        nc.sync.dma_start(wqk_t[:, 0:4], wqkr[:, 0:4])
        nc.sync.dma_start(x_t[:, 0:2, 0, 0:1024], xTr[:, 0:2, 0, 0:1024])
        nc.sync.dma_start(x_t[:, 0:2, 1, 0:1024], xTr[:, 0:2, 1, 0:1024])
        nc.sync.dma_start(wqk_t[:, 4:8], wqkr[:, 4:8])
        for dp in range(1, 4):
            nc.sync.dma_start(x_t[:, 2 * dp:2 * dp + 2, 0, 0:1024],
                              xTr[:, 2 * dp:2 * dp + 2, 0, 0:1024])
            nc.sync.dma_start(x_t[:, 2 * dp:2 * dp + 2, 1, 0:1024],
                              xTr[:, 2 * dp:2 * dp + 2, 1, 0:1024])
        nc.sync.dma_start(cos_t[:], cos4[:])
        nc.sync.dma_start(sin_t[:], sin4[:])
        nc.sync.dma_start(wv_t[:], wvr[:])
        for dp in range(4):
            for j in range(2):
                nc.sync.dma_start(x_t[:, 2 * dp:2 * dp + 2, j, 1024:2048],
                                  xTr[:, 2 * dp:2 * dp + 2, j, 1024:2048])
        for d2 in range(2):
            nc.sync.dma_start(wo_t[d2][:], woT[d2 * 128:(d2 + 1) * 128, :])

        # ============ Phase A: projections (all 4 chunks) ============
        with tc.tile_pool(name="ropet", bufs=4) as ropet:
            with tc.tile_pool(name="qkps", bufs=3, space="PSUM") as qkps, \
                 tc.tile_pool(name="vps", bufs=2, space="PSUM") as vps:
                for sc_i in range(4):
                    pools = (
                        lambda e, s=sc_i: qkps.tile(
                            [128, 512], F32, tag=f"qk{e % 2}",
                            name=f"qk{e}_{s}"),
                        lambda st: vps.tile([128, 256], F32, tag="vps",
                                            name=f"pv_{st}"),
                    )
                    # V of chunk 3 is deferred into the A3 attention loop
                    # (its PVs only need v_t[12..15] near the t-loop end).
                    for piece in _proj_chunk_pieces(
                            nc, sc_i, pools, x_t, wqk_t, wv_t, cos_t, sin_t,
                            qc, rstage, v_t, ropet, include_v=(sc_i < 3)):
                        piece()
                    if sc_i == 1:
                        # A3's first 8 k-tiles only need kc columns 0:1024
                        _scatter(nc, rstage, 2, kc, 0, 1024)
                    if sc_i == 3:
                        _scatter(nc, rstage, 2, kc, 1024, 2048)
                        _scatter(nc, rstage, 0, qc, 512, 1536)

            # ======== Phase B: attention quarters (3,2,1,0) + out-proj ====
            # Descending order puts the smallest quarter last (short tail);
            # the out-projection of each processed quarter becomes PE filler
            # work inside the next quarter's attention loop.
            # psum: sc [128,1024]x2 + xacc [65,512]x3 + ops [128,512]x1 = 8.
            # xacc is 3-deep so a new head-pair's accumulators never wait on
            # the previous pair's norm reads (the boundary serializer).
            with tc.tile_pool(name="attps", bufs=2, space="PSUM") as attps, \
                 tc.tile_pool(name="xaccps", bufs=2, space="PSUM") as xaccps, \
                 tc.tile_pool(name="ops", bufs=2, space="PSUM") as opsp:
                ops_alloc = (lambda st: opsp.tile([128, 512], F32, tag="ops",
                                                  name=f"dpv_{st}"))
                seq = [(0, 0), (0, 1), (3, 0), (3, 1), (2, 0), (2, 1),
                       (1, 0), (1, 1)]
                prestash = {}
                prev_qh = None
                for qh in (0, 3, 2, 1):
                    qlo, qhi = 512 * qh, 512 * (qh + 1)
                    t_end = 4 * (qh + 1)
                    # Per-hp filler schedules (tile -> pieces): deferred
                    # V-projections are EAGER (one per tile from t=0, their
                    # PVs consume them later in the same loop); out-proj
                    # units of the previous quarter are spread evenly.
                    sched = [{}, {}]
                    if qh == 3:
                        # all 8 pieces in hp0, positioned as late as each
                        # PV dependency allows (the diag region is where ACT
                        # outpaces PE and needs PE filler)
                        vp = list(_v_pieces(nc, range(12, 16), ops_alloc,
                                            x_t, wv_t, v_t))
                        vpos = [0, 8, 9, 10, 11, 12, 13, 14]
                        for i, p in enumerate(vp):
                            sched[0].setdefault(vpos[i], []).append(p)
                    if prev_qh is not None:
                        ou = list(_outproj_units(nc, opsp, ooutp, xn, wo_t,
                                                 out, prev_qh,
                                                 act_c1=False))
                        # one unit at the hp-boundary warmup, the rest in the
                        # trailing diag region where ACT outpaces PE
                        pos = [t_end - 4, t_end - 3, t_end - 2, t_end - 1]
                        for k in range(2):
                            for i, p in enumerate(ou[4 * k:4 * k + 4]):
                                sched[k].setdefault(pos[i], []).append(p)

                    for hp in range(2):
                        # last filler slot: prestart the NEXT pair's first
                        # score tile + exp so ACT never idles across seams
                        i = seq.index((qh, hp))
                        if i + 1 < len(seq):
                            nqh, nhp = seq[i + 1]
                            sched[hp].setdefault(t_end - 1, []).append(
                                lambda nqh=nqh, nhp=nhp: _prestart_t0(
                                    nc, attps, ptp, nqh, nhp, kc, qc,
                                    maskT, ident, prestash))
                        heads = (2 * hp, 2 * hp + 1)
                        xaccs = {}
                        for h in heads:
                            xaccs[h] = xaccps.tile([65, 512], F32, tag="xacc",
                                                   name=f"xacc_{qh}_{h}")
                        _attention_pair(nc, attps, ptp, heads, qlo, qhi,
                                        xaccs, kc, qc, v_t, maskT, ident,
                                        sched[hp],
                                        pend0=prestash.pop((qh, hp), None))
                        if qh == 1 and hp == 1:
                            # pre-start the tail's d2=0 accumulations (they
                            # only need xn[0]) so the PE overlaps the final
                            # norm ladder instead of waiting behind it
                            tail_p1, tail_p2 = _tail_outproj(
                                nc, attps, opsp, ooutp, xn, wo_t, out)
                            tail_p1()
                        for h in heads:
                            xacc = xaccs[h]
                            # stash the accumulators to SBUF on ACT first:
                            # this releases the xacc PSUM bank in ~0.6us so
                            # the next pair's PVs never wait on the norm
                            # ladder (which otherwise holds the bank ~5us -
                            # reciprocal on a [1,512] single-partition AP is
                            # ~3.3us on DVE).  The ladder then runs from
                            # SBUF fully off the PE critical path.
                            xs = nrmp.tile([65, 512], F32, tag="xs")
                            # copy on DVE: ACT (exp) is the busier engine
                            # in the attention steady state
                            nc.vector.tensor_scalar_mul(xs[:], xacc[:], 1.0)
                            xsrc = xs
                            # den must sit at partition base 0: the
                            # custom-DVE reciprocal_approx_fast corrupts
                            # data when its input AP has a non-zero base
                            # partition (verified on hw), so stage row 64
                            # down with a cheap DVE copy first.
                            den = nrmp.tile([1, 512], F32, tag="den")
                            nc.vector.tensor_scalar_mul(den[:], xsrc[64:65, :],
                                                        1.0)
                            recip = nrmp.tile([1, 512], F32, tag="recip")
                            nc.vector.reciprocal_approx_fast(recip[:], den[:])
                            rb = nrmp.tile([64, 512], F32, tag="rb")
                            nc.gpsimd.partition_broadcast(rb[:], recip[:])
                            dst = xn[h // 2][64 * (h % 2):64 * (h % 2) + 64, :]
                            nc.vector.tensor_tensor(
                                dst[:, qlo:qhi], xsrc[0:64, :], rb[:],
                                mybir.AluOpType.mult)
                    prev_qh = qh
                # tail: rest of the last quarter's out-projection
                tail_p2()


def _to_bf16(a):
    import ml_dtypes
    return np.ascontiguousarray(a.astype(ml_dtypes.bfloat16))


def _to_hilo8(a, scale):
    """[R, C] f32 -> [R, 2, C] fp8 e4m3 (hi, residual-lo) after scaling."""
    import ml_dtypes
    E4 = ml_dtypes.float8_e4m3
    a = np.asarray(a, np.float32) * scale
    hi = a.astype(E4)
    lo = (a - hi.astype(np.float32)).astype(E4)
    return np.ascontiguousarray(np.stack([hi, lo], axis=1))


def prepare_in_maps(inputs, positions, W_in, W_out):
    """Build per-core input shards (all host-side numpy prep)."""
    inputs = np.ascontiguousarray(inputs, dtype=np.float32)
    W_in = np.ascontiguousarray(W_in, dtype=np.float32)
    W_out = np.ascontiguousarray(W_out, dtype=np.float32)
    positions = np.asarray(positions)

    inv_freq = 1.0 / (ROPE_BASE ** (np.arange(0, HD, 2, dtype=np.float32) / HD))
    in_maps = []
    for core in range(N_CORES):
        b, g = divmod(core, G)
        heads = [g * HPG + h for h in range(HPG)]

        xTb = inputs[b].T                                          # [D, S]

        # RoPE tables [128, S]: rows = 4 copies of the 32 freqs
        ang = positions[b].astype(np.float32)[None, :] * inv_freq[:, None]
        cos4 = np.tile(np.cos(ang), (4, 1)).astype(np.float32)
        sin4 = np.tile(np.sin(ang), (4, 1)).astype(np.float32)

        # QK weight blocks: QA/QB/KA/KB, each 128 rows (4 heads x 32)
        def rows(base_off):
            idx = []
            for h in heads:
                idx.extend(h * 3 * HD + base_off + f for f in range(32))
            return idx
        qk_idx = rows(0) + rows(32) + rows(64) + rows(96)
        wqkT = W_in[qk_idx].T                                      # [D, 512]

        v_idx = []
        for h in heads:
            v_idx.extend(h * 3 * HD + 2 * HD + f for f in range(HD))
        wvT = W_in[v_idx].T                                        # [D, 256]

        # W_out columns for this group's feature slice, transposed
        dsl = [h * HD + f for h in heads for f in range(HD)]
        woT = W_out[:, dsl].T                                      # [256, D]

        in_maps.append({
            "xT": _to_hilo8(xTb, X_SCALE),
            "wqkT": _to_hilo8(wqkT, W_SCALE),
            "wvT": _to_hilo8(wvT, W_SCALE),
            "cos4": cos4, "sin4": sin4, "woT": _to_bf16(woT),
        })
    return in_maps


def assemble_output(results):
    """Sum the 4 per-group partials (bf16) for each batch."""
    out = np.zeros((B, S, D), dtype=np.float32)
    for core in range(N_CORES):
        b = core // G
        out[b] += np.asarray(results[core]["out"], dtype=np.float32)
    return out


_NC_CACHE = {}


def get_nc():
    if "nc" not in _NC_CACHE:
        _NC_CACHE["nc"] = build_nc()
    return _NC_CACHE["nc"]


def kernel(inputs, positions, W_in, W_out):
    nc = get_nc()
    in_maps = prepare_in_maps(inputs, positions, W_in, W_out)
    res = bass_utils.run_bass_kernel_spmd(
        nc, in_maps, core_ids=list(range(N_CORES)))
    return assemble_output(res.results)

